# revision 24
# baseline (speedup 1.0000x reference)
"""Multi-head causal self-attention (B=2, S=2048, D=1024, H=16) on 8 TRN2 cores.

Sharding: core c handles batch b = c//4 and head group g = c%4 (4 heads,
256 output dims). W_q/W_k/W_v are split column-wise per head group, W_o
row-wise; each core computes a partial [S, D] output product which the host
sums per batch (plus the (bv @ Wo.T + bo) row, exact because softmax rows
sum to 1).

v3 pipeline (PE-busy-driven rework of v2; baseline trace showed PE busy
133us of a 162us wall, with ~26us of exposed drains and ~19us of ACT time
spent on projection evacuations that blocked the PE filler chains):
  - ACT runs *only* the softmax exp stream (plus a table preload at t=0).
    All PSUM evacuations (Q/K bias-add, V copy, out-proj copy) moved to DVE
    (tensor_scalar_add with a [128,1] bias AP handles the bias broadcast).
  - Filler work (projections of later chunks, out-projection + normalize of
    earlier chunks) is emitted as fine-grained units paced fractionally
    across the attention j-loop, with per-chunk lists sized so the PE never
    starves while ACT grinds exp: c0 <- [V0,Q1,K1], c1 <- [V1,bc0,Q2,K2],
    c2 <- [V2,bc1,oproj0,Q3,K3], c3 <- [V3,bc2,oproj1,oproj2].
  - Q/K/V filler chains accumulate in 256-wide half-bank slices of one
    persistent PSUM tile, ping-ponging halves so a chain's DVE evacuation
    overlaps the next chain's matmuls instead of stalling them.
  - Chunk-0 Q/K projections stream against kc-granular DMA pieces of the
    first x chunk (weights on the gpsimd DMA queue, x on the sync queue),
    so the first exp lands ~9us after start instead of ~18us.
  - Scores matmuls row-pack 2 heads (K=64 at row groups 0/64); PV col-packs
    2 heads (M=64 at col groups 0/64); softmax denominators come from 4
    concurrent M=32 ones-matmuls at col groups 0/32/64/96.
  - Tail out-projection rotates PSUM banks (psPO/psSUM/psW) so evacuations
    overlap the remaining matmuls.
"""

import os
import sys

import numpy as np

# concourse (Bass/Tile) normally comes from PYTHONPATH; fall back to the
# container's copy when run from a bare directory.
for _p in ("/root/.axon_site/_ro/trn_rl_repo", "/opt/trn_rl_repo"):
    if _p not in sys.path and os.path.isdir(_p):
        sys.path.append(_p)

S = 2048
D = 1024
HL = 4          # heads per core
DL = 256        # local head dims per core
SC = 512        # sq chunk width
NSC = S // SC   # 4 chunks
KC = D // 128   # 8 contraction chunks for the projections

MM_DTYPE = os.environ.get("BASS_MM_DTYPE", "f16")
TRACE = os.environ.get("BASS_KERNEL_TRACE", "0") == "1"
DEBUG_TAPS = os.environ.get("BASS_KERNEL_DEBUG", "0") == "1"

_CACHE = {}


def _build():
    import concourse.bass as bass
    import concourse.mybir as mybir
    import concourse.tile as tile
    from concourse import bacc

    dt = mybir.dt
    f32 = dt.float32
    mmdt = {"f16": dt.float16, "f32r": dt.float32r, "f32": dt.float32}[MM_DTYPE]

    nc = bacc.Bacc("TRN2", target_bir_lowering=False, debug=False)

    # chunk 0 of q/k is kc-major so each single-kc piece DMA is a fully
    # linear dram read; later chunks (and all of v) are chunk-major dense
    # so each whole-chunk DMA is linear (4KB+ per partition, stride==run)
    xq0T = nc.dram_tensor("xq0T", [KC, 128, SC], mmdt, kind="ExternalInput").ap()
    xk0T = nc.dram_tensor("xk0T", [KC, 128, SC], mmdt, kind="ExternalInput").ap()
    xqRT = nc.dram_tensor("xqRT", [NSC - 1, 128, KC, SC], mmdt, kind="ExternalInput").ap()
    xkRT = nc.dram_tensor("xkRT", [NSC - 1, 128, KC, SC], mmdt, kind="ExternalInput").ap()
    xvT = nc.dram_tensor("xvT", [NSC, 128, KC, SC], mmdt, kind="ExternalInput").ap()
    wqT = nc.dram_tensor("wqT", [128, KC, DL], mmdt, kind="ExternalInput").ap()
    wkT = nc.dram_tensor("wkT", [128, KC, DL], mmdt, kind="ExternalInput").ap()
    wvT = nc.dram_tensor("wvT", [128, KC, DL], mmdt, kind="ExternalInput").ap()
    woT = nc.dram_tensor("woT", [128, 2, D], mmdt, kind="ExternalInput").ap()
    bqd = nc.dram_tensor("bqd", [128, 2], f32, kind="ExternalInput").ap()
    bkd = nc.dram_tensor("bkd", [128, 2], f32, kind="ExternalInput").ap()
    maskd = nc.dram_tensor("maskd", [128, 128], mmdt, kind="ExternalInput").ap()
    outd = nc.dram_tensor("out", [S, D], mmdt, kind="ExternalOutput").ap()
    if DEBUG_TAPS:
        dbg_qt = nc.dram_tensor("dbg_qt", [2, 128, S], mmdt, kind="ExternalOutput").ap()
        dbg_kt = nc.dram_tensor("dbg_kt", [2, 128, S], mmdt, kind="ExternalOutput").ap()
        dbg_vt = nc.dram_tensor("dbg_vt", [128, 16, 256], mmdt, kind="ExternalOutput").ap()
        dbg_otn = nc.dram_tensor("dbg_otn", [2, 128, S], mmdt, kind="ExternalOutput").ap()

    Exp = mybir.ActivationFunctionType.Exp

    def mm(ps, lhsT, rhs, start, stop, tile_position=None, skip=False):
        nc.tensor.matmul(
            ps, lhsT, rhs, start=start, stop=stop, tile_position=tile_position,
            skip_group_check=skip,
        )

    with tile.TileContext(nc) as tc:
        with (
            tc.tile_pool(name="const", bufs=1) as constp,
            tc.tile_pool(name="w", bufs=1) as wp,
            tc.tile_pool(name="x", bufs=10) as xp,
            tc.tile_pool(name="pp", bufs=1) as pp,
            tc.tile_pool(name="pt", bufs=4) as ptp,
            tc.tile_pool(name="otr", bufs=4) as orp,
            tc.tile_pool(name="rs", bufs=2) as rsp,
            tc.tile_pool(name="osb", bufs=4) as osp,
            tc.tile_pool(name="psS", bufs=2, space="PSUM") as psS,
            tc.tile_pool(name="psPO", bufs=2, space="PSUM") as psPO,
            tc.tile_pool(name="psSUM", bufs=1, space="PSUM") as psSUM,
            tc.tile_pool(name="psW", bufs=1, space="PSUM") as psW,
        ):
            # ---- DVE constants (first so warmup + exp preload can start) ----
            ones_f32 = constp.tile([128, 64], f32, tag="ones_f32")
            nc.vector.memset(ones_f32[:], 1.0)
            ones_sb = constp.tile([128, 64], mmdt, tag="ones")
            nc.vector.tensor_copy(ones_sb[:], ones_f32[:])
            wz = constp.tile([128, 512], mmdt, tag="wz")
            nc.vector.memset(wz[:], 0.0)

            # ---- DMA ordering: only the critical preamble bytes (wq, xq0,
            # wk, xk0 — split across the scalar and sync queues for ~2x
            # bandwidth) precede everything else; total DMA bandwidth
            # (~240 GB/s across queues) is the startup constraint ----
            mask_sb = constp.tile([128, 1, 128], mmdt, tag="mask")
            nc.scalar.dma_start(mask_sb[:, 0, :], maskd[:])
            bq_sb = constp.tile([128, 2], f32, tag="bq")
            nc.scalar.dma_start(bq_sb[:], bqd[:])
            bk_sb = constp.tile([128, 2], f32, tag="bk")
            nc.scalar.dma_start(bk_sb[:], bkd[:])
            wq_sb = wp.tile([128, KC, DL], mmdt, tag="wq")
            nc.scalar.dma_start(wq_sb[:], wqT[:])

            xq0 = xp.tile([128, KC, SC], mmdt, tag="x", name="xq0")
            for kc in range(KC):
                eng = nc.sync if kc % 2 == 0 else nc.scalar
                eng.dma_start(xq0[:, kc, :], xq0T[kc])
            wk_sb = wp.tile([128, KC, DL], mmdt, tag="wk")
            nc.scalar.dma_start(wk_sb[:], wkT[:])
            xk0 = xp.tile([128, KC, SC], mmdt, tag="x", name="xk0")
            for kc in range(KC):
                eng = nc.sync if kc % 2 == 0 else nc.scalar
                eng.dma_start(xk0[:, kc, :], xk0T[kc])

            # exp table preload: after the critical scalar-queue DMA issues
            # (so it doesn't delay them) but well before the first real exp
            pre_sb = constp.tile([128, 8], mmdt, tag="pre")
            nc.scalar.activation(pre_sb[:], ones_f32[:, 0:8], Exp)

            # non-critical loads, consumption order, balanced across queues
            xv0 = xp.tile([128, KC, SC], mmdt, tag="x", name="xv0")
            nc.sync.dma_start(xv0[:], xvT[0])
            wv_sb = wp.tile([128, KC, DL], mmdt, tag="wv")
            nc.scalar.dma_start(wv_sb[:], wvT[:])

            xts_all = {0: [xq0, xk0, xv0]}
            qs = {"q": nc.sync, "k": nc.scalar, "v": nc.sync}
            srcs = {"q": xqRT, "k": xkRT}
            for cc in range(1, NSC):
                xts = []
                for nm in ("q", "k", "v"):
                    t = xp.tile([128, KC, SC], mmdt, tag="x", name=f"x{nm}{cc}")
                    src = xvT[cc] if nm == "v" else srcs[nm][cc - 1]
                    qs[nm].dma_start(t[:], src)
                    xts.append(t)
                xts_all[cc] = xts
            wo_sb = wp.tile([128, 2, D], mmdt, tag="wo")
            nc.scalar.dma_start(wo_sb[:], woT[:])

            # ---- persistent PSUM accumulator for filler chains + warmup ----
            psw = psW.tile([128, 512], f32, tag="psw", name="psw")
            for _ in range(8):
                mm(psw[:], wz[:, 0:128], wz[:], start=True, stop=True, skip=True)

            # ---- persistent activations ----
            QT = [pp.tile([128, S], mmdt, tag=f"qt{t}", name=f"qt{t}") for t in range(2)]
            KT = [pp.tile([128, S], mmdt, tag=f"kt{t}", name=f"kt{t}") for t in range(2)]
            Vt = pp.tile([128, 16, 64 * HL], mmdt, tag="vt")
            OTn = [pp.tile([128, S], mmdt, tag=f"otn{t}", name=f"otn{t}") for t in range(2)]

            # ---- chunk-0 Q/K projections: kc-major streaming into psS ----
            def preamble_proj(xt, w_sb, b_sb, dstT):
                # One accumulation group per PSUM bank (group = t): start
                # pending-zeroes the whole 2KB zero region, so groups must
                # not interleave within a bank. Within a group the two
                # 256-wide h-subregions may interleave freely.
                ps = psS.tile([128, 1024], f32, tag="pss", name="pre_proj")
                for kc in range(KC):
                    for t in range(2):
                        for h in range(2):
                            mm(
                                ps[:, t * 512 + h * 256 : t * 512 + h * 256 + 256],
                                w_sb[:, kc, t * 128 : (t + 1) * 128],
                                xt[:, kc, h * 256 : h * 256 + 256],
                                start=(kc == 0 and h == 0),
                                stop=(kc == KC - 1 and h == 1),
                                skip=True,
                            )
                for t in range(2):
                    for h in range(2):
                        nc.vector.tensor_scalar_add(
                            dstT[t][:, h * 256 : h * 256 + 256],
                            ps[:, t * 512 + h * 256 : t * 512 + h * 256 + 256],
                            b_sb[:, t : t + 1],
                        )

            preamble_proj(xq0, wq_sb, bq_sb, QT)
            preamble_proj(xk0, wk_sb, bk_sb, KT)

            # ---- filler units (each: a few MMs + a DVE evacuation) ----
            whalf = [0]

            def take_half():
                h = whalf[0]
                whalf[0] ^= 1
                return psw[:, h * 256 : h * 256 + 256]

            def proj_qk_unit(c2, xt, w_sb, b_sb, dstT, t, h):
                ph = take_half()
                for kc in range(KC):
                    mm(
                        ph,
                        w_sb[:, kc, t * 128 : (t + 1) * 128],
                        xt[:, kc, h * 256 : h * 256 + 256],
                        start=(kc == 0),
                        stop=(kc == KC - 1),
                        skip=True,
                    )
                nc.vector.tensor_scalar_add(
                    dstT[t][:, c2 * SC + h * 256 : c2 * SC + h * 256 + 256],
                    ph,
                    b_sb[:, t : t + 1],
                )

            def proj_v_unit(c2, xt, s):
                ph = take_half()
                for kc in range(KC):
                    mm(
                        ph,
                        xt[:, kc, s * 128 : (s + 1) * 128],
                        wv_sb[:, kc, :],
                        start=(kc == 0),
                        stop=(kc == KC - 1),
                        skip=True,
                    )
                nc.vector.tensor_copy(Vt[:, 4 * c2 + s, :], ph)

            def oproj_unit(st, n):
                # full-bank unit; scheduled at most one per filler position
                # so the previous unit's DVE evacuation lands in time
                for k2 in range(2):
                    mm(
                        psw[:],
                        OTn[k2][:, st * 128 : (st + 1) * 128],
                        wo_sb[:, k2, n * 512 : (n + 1) * 512],
                        start=(k2 == 0),
                        stop=(k2 == 1),
                        skip=True,
                    )
                osb = osp.tile([128, 512], mmdt, tag="osb")
                nc.vector.tensor_copy(osb[:], psw[:])
                nc.sync.dma_start(
                    outd[st * 128 : (st + 1) * 128, n * 512 : (n + 1) * 512], osb[:]
                )

            def oproj_tail(st, n, pool, evac_act):
                ps = pool.tile([128, 512], f32, tag=pool_tag(pool), name=f"op{st}_{n}")
                for k2 in range(2):
                    mm(
                        ps[:],
                        OTn[k2][:, st * 128 : (st + 1) * 128],
                        wo_sb[:, k2, n * 512 : (n + 1) * 512],
                        start=(k2 == 0),
                        stop=(k2 == 1),
                        skip=True,
                    )
                osb = osp.tile([128, 512], mmdt, tag="osb")
                if evac_act:
                    nc.scalar.copy(osb[:], ps[:])
                else:
                    nc.vector.tensor_copy(osb[:], ps[:])
                nc.sync.dma_start(
                    outd[st * 128 : (st + 1) * 128, n * 512 : (n + 1) * 512], osb[:]
                )

            def pool_tag(pool):
                return {id(psS): "pss", id(psPO): "po", id(psSUM): "sums", id(psW): "psw"}[id(pool)]

            def bc_unit(c2, t, otrs, rr):
                for h2 in range(2):
                    head = 2 * t + h2
                    mm(
                        psw[h2 * 64 : h2 * 64 + 64, :],
                        ones_sb[32 * head : 32 * head + 1, :],
                        rr[32 * head : 32 * head + 1, :],
                        start=True,
                        stop=True,
                        tile_position=(32 * head, h2 * 64),
                        skip=True,
                    )
                nc.vector.tensor_mul(
                    OTn[t][:, c2 * SC : (c2 + 1) * SC], otrs[t][:], psw[:]
                )

            # ---- attention emission helpers, split per head-pair t so the
            # emission order matches the exp-stream release order ----
            def emit_scores_t(c, j, t):
                d = j - 4 * c
                x0 = max(0, 128 * d)
                ps = psS.tile([128, 1024], f32, tag="pss")
                for h2 in range(2):
                    p0 = h2 * 64
                    mm(
                        ps[:, h2 * 512 + x0 : (h2 + 1) * 512],
                        KT[t][p0 : p0 + 64, j * 128 : (j + 1) * 128],
                        QT[t][p0 : p0 + 64, c * SC + x0 : (c + 1) * SC],
                        start=True,
                        stop=True,
                    )
                pt = ptp.tile([128, 1024], mmdt, tag="pt")
                psv = ps.rearrange("p (h x) -> p h x", x=512)
                ptv = pt.rearrange("p (h x) -> p h x", x=512)
                nc.scalar.activation(ptv[:, :, x0:], psv[:, :, x0:], Exp)
                if d >= 0:
                    nc.vector.tensor_mul(
                        ptv[:, :, x0 : x0 + 128],
                        ptv[:, :, x0 : x0 + 128],
                        mask_sb[:, 0:1, :].broadcast_to([128, 2, 128]),
                    )
                return pt

            def emit_pv_t(c, j, jmax, pt, po, t):
                d = j - 4 * c
                x0 = max(0, 128 * d)
                for h2 in range(2):
                    h = 2 * t + h2
                    mm(
                        po[t][h2 * 64 : h2 * 64 + 64, x0:],
                        Vt[:, j, h * 64 : (h + 1) * 64],
                        pt[:, h2 * 512 + x0 : (h2 + 1) * 512],
                        start=(j == 0),
                        stop=(j == jmax),
                        skip=True,
                    )

            def emit_sums(c, j, jmax, pts, sums):
                d = j - 4 * c
                x0 = max(0, 128 * d)
                for h in range(HL):
                    t, h2 = divmod(h, 2)
                    mm(
                        sums[32 * h : 32 * h + 32, x0:],
                        ones_sb[:, 0:32],
                        pts[t][:, h2 * 512 + x0 : (h2 + 1) * 512],
                        start=(j == 0),
                        stop=(j == jmax),
                        tile_position=(0, 32 * h),
                        skip=True,
                    )

            def chunk_end(c, po, sums, last=False):
                def recip_chain():
                    rf = rsp.tile([128, 512], f32, tag="rf", name=f"rf{c}")
                    nc.vector.reciprocal_approx_fast(rf[:], sums[:])
                    rr = rsp.tile([128, 512], mmdt, tag="rr", name=f"rr{c}")
                    nc.vector.tensor_copy(rr[:], rf[:])
                    return rr

                def po_evac():
                    otrs = []
                    for t in range(2):
                        otr = orp.tile([128, 512], f32, tag="otr", name=f"otr{c}_{t}")
                        nc.vector.tensor_copy(otr[:], po[t][:])
                        otrs.append(otr)
                    return otrs

                if last:
                    rr = recip_chain()
                    otrs = po_evac()
                else:
                    otrs = po_evac()
                    rr = recip_chain()
                return otrs, rr

            # ---- per-chunk filler unit lists ----
            def qk_units(c2):
                xq, xk, _ = xts_all[c2]
                us = [
                    (lambda t=t, h=h, xq=xq, c2=c2: proj_qk_unit(c2, xq, wq_sb, bq_sb, QT, t, h))
                    for t in range(2) for h in range(2)
                ] + [
                    (lambda t=t, h=h, xk=xk, c2=c2: proj_qk_unit(c2, xk, wk_sb, bk_sb, KT, t, h))
                    for t in range(2) for h in range(2)
                ]
                return us

            def v_units(c2):
                xv = xts_all[c2][2]
                return [
                    (lambda s=s, xv=xv, c2=c2: proj_v_unit(c2, xv, s)) for s in range(4)
                ]

            def bc_units(c2, ep):
                otrs, rr = ep
                return [
                    (lambda t=t: bc_unit(c2, t, otrs, rr)) for t in range(2)
                ]

            def oproj_units(c2):
                return [
                    (lambda st=st, n=n: oproj_unit(st, n))
                    for st in range(4 * c2, 4 * c2 + 4) for n in range(2)
                ]

            # ---- main fused loop ----
            # Per-iteration emission order matches the release order of the
            # exp stream: scores(j+1,t) frees up as soon as exp(j,t) has read
            # its PSUM tile, and pv(j,t) as soon as exp(j,t)+mask are done,
            # so [S(t0), F, P(t0), S(t1), F, P(t1), SUMS] keeps the PE FIFO
            # head unblocked with filler units absorbing the exp latency.
            ep_states = {}
            pts_cur = None
            for c in range(NSC):
                jmax = 4 * c + 3
                nj = jmax + 1
                units = list(v_units(c))
                if c >= 1:
                    units += bc_units(c - 1, ep_states[c - 1])
                if c == 2:
                    units += oproj_units(0)
                if c == 3:
                    units += oproj_units(1) + oproj_units(2)
                if c + 1 < NSC:
                    units += qk_units(c + 1)

                po = [
                    psPO.tile([128, 512], f32, tag="po", name=f"po{c}_{t}")
                    for t in range(2)
                ]
                sums = psSUM.tile([128, 512], f32, tag="sums", name=f"sums{c}")

                pace = len(units) / nj
                acc = 0.0
                popped = 0
                if pts_cur is None:
                    pts_cur = [emit_scores_t(c, 0, 0), emit_scores_t(c, 0, 1)]
                for j in range(nj):
                    acc += pace
                    npop = int(acc)
                    acc -= npop
                    # V(c) units head the list; slot s=j-4c must be written
                    # before this iteration's pv reads Vt[:, j].
                    d = j - 4 * c
                    while d >= 0 and popped <= d and units:
                        units.pop(0)()
                        popped += 1
                        npop = max(0, npop - 1)
                    flush = j == nj - 1
                    if flush:
                        # All remaining units must precede the next chunk's
                        # scores: Tile dependencies follow program order, and
                        # the Q(c+1) units write the QT region that
                        # scores(c+1, 0) reads.
                        while units:
                            units.pop(0)()
                            popped += 1
                    elif npop >= 1 and units:
                        units.pop(0)()
                        popped += 1
                        npop -= 1
                    nxt = (c, j + 1) if j < jmax else (
                        (c + 1, 0) if c + 1 < NSC else None
                    )
                    pts_new = [None, None]
                    if nxt:
                        pts_new[0] = emit_scores_t(nxt[0], nxt[1], 0)
                        pts_new[1] = emit_scores_t(nxt[0], nxt[1], 1)
                    while not flush and npop > 0 and units:
                        units.pop(0)()
                        popped += 1
                        npop -= 1
                    emit_pv_t(c, j, jmax, pts_cur[0], po, 0)
                    emit_pv_t(c, j, jmax, pts_cur[1], po, 1)
                    emit_sums(c, j, jmax, pts_cur, sums)
                    pts_cur = pts_new
                ep_states[c] = chunk_end(c, po, sums, last=(c == NSC - 1))

            # ---- tail: last chunk's normalize + out-projection ----
            otrs3, rr3 = ep_states[NSC - 1]
            for t in range(2):
                bc_unit(NSC - 1, t, otrs3, rr3)
            tail_pools = [psPO, psSUM, psPO, psS]
            i = 0
            for st in range(4 * (NSC - 1), 4 * (NSC - 1) + 4):
                for n in range(2):
                    oproj_tail(st, n, tail_pools[i % 4], evac_act=(i % 2 == 0))
                    i += 1

            if DEBUG_TAPS:
                for t in range(2):
                    nc.sync.dma_start(dbg_qt[t], QT[t][:])
                    nc.sync.dma_start(dbg_kt[t], KT[t][:])
                    nc.sync.dma_start(dbg_otn[t], OTn[t][:])
                nc.sync.dma_start(dbg_vt[:], Vt[:])

    nc.compile()
    return nc


def _get_nc():
    key = ("nc", MM_DTYPE)
    if key not in _CACHE:
        _CACHE[key] = _build()
    return _CACHE[key]


def _warr(wT, n):
    """[K, n] -> [128, K//128, n] so the device DMA is contiguous."""
    K = wT.shape[0]
    return np.ascontiguousarray(wT.reshape(K // 128, 128, n).transpose(1, 0, 2))


def make_in_maps(q, k, v, Wq, bq, Wk, bk, Wv, bv, Wo, bo):
    """Host-side shard prep: per-core input dict."""
    f32 = np.float32
    md = {"f16": np.float16, "f32r": f32, "f32": f32}[MM_DTYPE]
    mask = (np.arange(128)[None, :] >= np.arange(128)[:, None]).astype(md)
    # per-batch transposes shared by the 4 cores of each batch.
    # chunk 0 of q/k is [KC, 128, SC] (kc-major: single-kc DMA pieces are
    # linear dram reads); chunks 1..3 and all of v are [c, 128, KC, SC]
    # (dense per chunk: whole-chunk DMAs are linear).
    def _x0arr(x):
        a = x.T.astype(md).reshape(KC, 128, NSC, SC)
        return np.ascontiguousarray(a[:, :, 0, :])

    def _xRarr(x, c0):
        a = x.T.astype(md).reshape(KC, 128, NSC, SC).transpose(2, 1, 0, 3)
        return np.ascontiguousarray(a[c0:])

    xq0T = [_x0arr(q[b]) for b in range(2)]
    xk0T = [_x0arr(k[b]) for b in range(2)]
    xqRT = [_xRarr(q[b], 1) for b in range(2)]
    xkRT = [_xRarr(k[b], 1) for b in range(2)]
    xvT = [_xRarr(v[b], 0) for b in range(2)]
    in_maps = []
    for c in range(8):
        b, g = c // 4, c % 4
        sl = slice(DL * g, DL * (g + 1))
        in_maps.append(
            {
                "xq0T": xq0T[b],
                "xk0T": xk0T[b],
                "xqRT": xqRT[b],
                "xkRT": xkRT[b],
                "xvT": xvT[b],
                "wqT": _warr((Wq[sl, :].T * f32(0.125)).astype(md), DL),
                "wkT": _warr(Wk[sl, :].T.astype(md), DL),
                "wvT": _warr(Wv[sl, :].T.astype(md), DL),
                "woT": _warr(Wo[:, sl].T.astype(md), D),
                "bqd": np.ascontiguousarray((bq[sl] * f32(0.125)).reshape(2, 128).T),
                "bkd": np.ascontiguousarray(bk[sl].reshape(2, 128).T),
                "maskd": mask,
            }
        )
    return in_maps


def kernel(q, k, v, Wq, bq, Wk, bk, Wv, bv, Wo, bo):
    from concourse.bass_utils import run_bass_kernel_spmd

    args = [np.asarray(a, dtype=np.float32) for a in (q, k, v, Wq, bq, Wk, bk, Wv, bv, Wo, bo)]
    q, k, v, Wq, bq, Wk, bk, Wv, bv, Wo, bo = args
    nc = _get_nc()
    in_maps = make_in_maps(q, k, v, Wq, bq, Wk, bk, Wv, bv, Wo, bo)
    tmpdir = os.environ.get("BASS_KERNEL_TMPDIR") or None
    res = run_bass_kernel_spmd(nc, in_maps, list(range(8)), trace=TRACE, tmpdir=tmpdir)
    if TRACE and res.exec_time_ns is not None:
        print(f"HW exec time: {res.exec_time_ns} ns")
        print(f"HW exec time mean: {res.mean_exec_time_ns} ns")
    out = np.zeros((2, S, D), np.float32)
    for c in range(8):
        out[c // 4] += res.results[c]["out"].astype(np.float32)
    out += (bv @ Wo.T + bo)[None, None, :]
    return out


# revision 25
# speedup vs baseline: 1.0474x; 1.0474x over previous
"""Multi-head causal self-attention (B=2, S=2048, D=1024, H=16) on 8 TRN2 cores.

Sharding: core c handles batch b = c//4 and head group g = c%4 (4 heads,
256 output dims). W_q/W_k/W_v are split column-wise per head group, W_o
row-wise; each core computes a partial [S, D] output product which the host
sums per batch (plus the (bv @ Wo.T + bo) row, exact because softmax rows
sum to 1).

v3 pipeline (PE-busy-driven rework of v2; baseline trace showed PE busy
133us of a 162us wall, with ~26us of exposed drains and ~19us of ACT time
spent on projection evacuations that blocked the PE filler chains):
  - ACT runs *only* the softmax exp stream (plus a table preload at t=0).
    All PSUM evacuations (Q/K bias-add, V copy, out-proj copy) moved to DVE
    (tensor_scalar_add with a [128,1] bias AP handles the bias broadcast).
  - Filler work (projections of later chunks, out-projection + normalize of
    earlier chunks) is emitted as fine-grained units paced fractionally
    across the attention j-loop, with per-chunk lists sized so the PE never
    starves while ACT grinds exp: c0 <- [V0,Q1,K1], c1 <- [V1,bc0,Q2,K2],
    c2 <- [V2,bc1,oproj0,Q3,K3], c3 <- [V3,bc2,oproj1,oproj2].
  - Q/K/V filler chains accumulate in 256-wide half-bank slices of one
    persistent PSUM tile, ping-ponging halves so a chain's DVE evacuation
    overlaps the next chain's matmuls instead of stalling them.
  - Chunk-0 Q/K projections stream against kc-granular DMA pieces of the
    first x chunk (weights on the gpsimd DMA queue, x on the sync queue),
    so the first exp lands ~9us after start instead of ~18us.
  - Scores matmuls row-pack 2 heads (K=64 at row groups 0/64); PV col-packs
    2 heads (M=64 at col groups 0/64); softmax denominators come from 4
    concurrent M=32 ones-matmuls at col groups 0/32/64/96.
  - Tail out-projection rotates PSUM banks (psPO/psSUM/psW) so evacuations
    overlap the remaining matmuls.
"""

import os
import sys

import numpy as np

# concourse (Bass/Tile) normally comes from PYTHONPATH; fall back to the
# container's copy when run from a bare directory.
for _p in ("/root/.axon_site/_ro/trn_rl_repo", "/opt/trn_rl_repo"):
    if _p not in sys.path and os.path.isdir(_p):
        sys.path.append(_p)

S = 2048
D = 1024
HL = 4          # heads per core
DL = 256        # local head dims per core
SC = 512        # sq chunk width
NSC = S // SC   # 4 chunks
KC = D // 128   # 8 contraction chunks for the projections

MM_DTYPE = os.environ.get("BASS_MM_DTYPE", "f16")
TRACE = os.environ.get("BASS_KERNEL_TRACE", "0") == "1"
DEBUG_TAPS = os.environ.get("BASS_KERNEL_DEBUG", "0") == "1"

_CACHE = {}


def _build():
    import concourse.bass as bass
    import concourse.mybir as mybir
    import concourse.tile as tile
    from concourse import bacc

    dt = mybir.dt
    f32 = dt.float32
    mmdt = {"f16": dt.float16, "f32r": dt.float32r, "f32": dt.float32}[MM_DTYPE]

    nc = bacc.Bacc("TRN2", target_bir_lowering=False, debug=False)

    # chunk 0 of q/k is kc-major so each single-kc piece DMA is a fully
    # linear dram read; later chunks (and all of v) are chunk-major dense
    # so each whole-chunk DMA is linear (4KB+ per partition, stride==run)
    xq0T = nc.dram_tensor("xq0T", [KC, 128, SC], mmdt, kind="ExternalInput").ap()
    xk0T = nc.dram_tensor("xk0T", [KC, 128, SC], mmdt, kind="ExternalInput").ap()
    xqRT = nc.dram_tensor("xqRT", [NSC - 1, 128, KC, SC], mmdt, kind="ExternalInput").ap()
    xkRT = nc.dram_tensor("xkRT", [NSC - 1, 128, KC, SC], mmdt, kind="ExternalInput").ap()
    xvT = nc.dram_tensor("xvT", [NSC, 128, KC, SC], mmdt, kind="ExternalInput").ap()
    wqT = nc.dram_tensor("wqT", [128, KC, DL], mmdt, kind="ExternalInput").ap()
    wkT = nc.dram_tensor("wkT", [128, KC, DL], mmdt, kind="ExternalInput").ap()
    wvT = nc.dram_tensor("wvT", [128, KC, DL], mmdt, kind="ExternalInput").ap()
    woT = nc.dram_tensor("woT", [128, 2, D], mmdt, kind="ExternalInput").ap()
    bqd = nc.dram_tensor("bqd", [128, 2], f32, kind="ExternalInput").ap()
    bkd = nc.dram_tensor("bkd", [128, 2], f32, kind="ExternalInput").ap()
    maskd = nc.dram_tensor("maskd", [128, 128], mmdt, kind="ExternalInput").ap()
    outd = nc.dram_tensor("out", [S, D], mmdt, kind="ExternalOutput").ap()
    if DEBUG_TAPS:
        dbg_qt = nc.dram_tensor("dbg_qt", [2, 128, S], mmdt, kind="ExternalOutput").ap()
        dbg_kt = nc.dram_tensor("dbg_kt", [2, 128, S], mmdt, kind="ExternalOutput").ap()
        dbg_vt = nc.dram_tensor("dbg_vt", [128, 16, 256], mmdt, kind="ExternalOutput").ap()
        dbg_otn = nc.dram_tensor("dbg_otn", [2, 128, S], mmdt, kind="ExternalOutput").ap()

    Exp = mybir.ActivationFunctionType.Exp

    def mm(ps, lhsT, rhs, start, stop, tile_position=None, skip=False):
        nc.tensor.matmul(
            ps, lhsT, rhs, start=start, stop=stop, tile_position=tile_position,
            skip_group_check=skip,
        )

    with tile.TileContext(nc) as tc:
        with (
            tc.tile_pool(name="const", bufs=1) as constp,
            tc.tile_pool(name="w", bufs=1) as wp,
            tc.tile_pool(name="x", bufs=10) as xp,
            tc.tile_pool(name="pp", bufs=1) as pp,
            tc.tile_pool(name="pt", bufs=4) as ptp,
            tc.tile_pool(name="otr", bufs=4) as orp,
            tc.tile_pool(name="rs", bufs=2) as rsp,
            tc.tile_pool(name="osb", bufs=4) as osp,
            tc.tile_pool(name="psS", bufs=2, space="PSUM") as psS,
            tc.tile_pool(name="psPO", bufs=2, space="PSUM") as psPO,
            tc.tile_pool(name="psSUM", bufs=1, space="PSUM") as psSUM,
            tc.tile_pool(name="psW", bufs=1, space="PSUM") as psW,
        ):
            # ---- DVE constants (first so warmup + exp preload can start) ----
            ones_f32 = constp.tile([128, 64], f32, tag="ones_f32")
            nc.vector.memset(ones_f32[:], 1.0)
            ones_sb = constp.tile([128, 64], mmdt, tag="ones")
            nc.vector.tensor_copy(ones_sb[:], ones_f32[:])
            wz = constp.tile([128, 512], mmdt, tag="wz")
            nc.vector.memset(wz[:], 0.0)

            # ---- DMA ordering (v5 scheme): weights on the scalar queue,
            # all x on the sync queue; chunk-0 q/k as single-kc linear
            # pieces so the preamble projections stream ----
            wq_sb = wp.tile([128, KC, DL], mmdt, tag="wq")
            nc.scalar.dma_start(wq_sb[:], wqT[:])
            bq_sb = constp.tile([128, 2], f32, tag="bq")
            nc.scalar.dma_start(bq_sb[:], bqd[:])
            wk_sb = wp.tile([128, KC, DL], mmdt, tag="wk")
            nc.scalar.dma_start(wk_sb[:], wkT[:])
            bk_sb = constp.tile([128, 2], f32, tag="bk")
            nc.scalar.dma_start(bk_sb[:], bkd[:])

            xq0 = xp.tile([128, KC, SC], mmdt, tag="x", name="xq0")
            for kc in range(KC):
                nc.sync.dma_start(xq0[:, kc, :], xq0T[kc])
            xk0 = xp.tile([128, KC, SC], mmdt, tag="x", name="xk0")
            for kc in range(KC):
                nc.sync.dma_start(xk0[:, kc, :], xk0T[kc])

            # exp table preload: after the critical scalar-queue DMA issues
            # (so it doesn't delay them) but well before the first real exp
            pre_sb = constp.tile([128, 8], mmdt, tag="pre")
            nc.scalar.activation(pre_sb[:], ones_f32[:, 0:8], Exp)

            wv_sb = wp.tile([128, KC, DL], mmdt, tag="wv")
            nc.scalar.dma_start(wv_sb[:], wvT[:])
            mask_sb = constp.tile([128, 1, 128], mmdt, tag="mask")
            nc.scalar.dma_start(mask_sb[:, 0, :], maskd[:])
            wo_sb = wp.tile([128, 2, D], mmdt, tag="wo")
            nc.scalar.dma_start(wo_sb[:], woT[:])

            xv0 = xp.tile([128, KC, SC], mmdt, tag="x", name="xv0")
            nc.sync.dma_start(xv0[:], xvT[0])
            xts_all = {0: [xq0, xk0, xv0]}
            srcs = {"q": xqRT, "k": xkRT}
            for cc in range(1, NSC):
                xts = []
                for nm in ("q", "k", "v"):
                    t = xp.tile([128, KC, SC], mmdt, tag="x", name=f"x{nm}{cc}")
                    src = xvT[cc] if nm == "v" else srcs[nm][cc - 1]
                    nc.sync.dma_start(t[:], src)
                    xts.append(t)
                xts_all[cc] = xts

            # ---- persistent PSUM accumulator for filler chains + warmup ----
            psw = psW.tile([128, 512], f32, tag="psw", name="psw")
            for _ in range(8):
                mm(psw[:], wz[:, 0:128], wz[:], start=True, stop=True, skip=True)

            # ---- persistent activations ----
            QT = [pp.tile([128, S], mmdt, tag=f"qt{t}", name=f"qt{t}") for t in range(2)]
            KT = [pp.tile([128, S], mmdt, tag=f"kt{t}", name=f"kt{t}") for t in range(2)]
            Vt = pp.tile([128, 16, 64 * HL], mmdt, tag="vt")
            OTn = [pp.tile([128, S], mmdt, tag=f"otn{t}", name=f"otn{t}") for t in range(2)]

            # ---- chunk-0 Q/K projections: kc-major streaming into psS ----
            def preamble_proj(xt, w_sb, b_sb, dstT):
                # One accumulation group per PSUM bank (group = t): start
                # pending-zeroes the whole 2KB zero region, so groups must
                # not interleave within a bank. Within a group the two
                # 256-wide h-subregions may interleave freely.
                ps = psS.tile([128, 1024], f32, tag="pss", name="pre_proj")
                for kc in range(KC):
                    for t in range(2):
                        for h in range(2):
                            mm(
                                ps[:, t * 512 + h * 256 : t * 512 + h * 256 + 256],
                                w_sb[:, kc, t * 128 : (t + 1) * 128],
                                xt[:, kc, h * 256 : h * 256 + 256],
                                start=(kc == 0 and h == 0),
                                stop=(kc == KC - 1 and h == 1),
                                skip=True,
                            )
                for t in range(2):
                    for h in range(2):
                        nc.vector.tensor_scalar_add(
                            dstT[t][:, h * 256 : h * 256 + 256],
                            ps[:, t * 512 + h * 256 : t * 512 + h * 256 + 256],
                            b_sb[:, t : t + 1],
                        )

            preamble_proj(xq0, wq_sb, bq_sb, QT)
            preamble_proj(xk0, wk_sb, bk_sb, KT)

            # ---- filler units (each: a few MMs + a DVE evacuation) ----
            whalf = [0]

            def take_half():
                h = whalf[0]
                whalf[0] ^= 1
                return psw[:, h * 256 : h * 256 + 256]

            def proj_qk_unit(c2, xt, w_sb, b_sb, dstT, t, h):
                ph = take_half()
                for kc in range(KC):
                    mm(
                        ph,
                        w_sb[:, kc, t * 128 : (t + 1) * 128],
                        xt[:, kc, h * 256 : h * 256 + 256],
                        start=(kc == 0),
                        stop=(kc == KC - 1),
                        skip=True,
                    )
                nc.vector.tensor_scalar_add(
                    dstT[t][:, c2 * SC + h * 256 : c2 * SC + h * 256 + 256],
                    ph,
                    b_sb[:, t : t + 1],
                )

            def proj_v_unit(c2, xt, s):
                ph = take_half()
                for kc in range(KC):
                    mm(
                        ph,
                        xt[:, kc, s * 128 : (s + 1) * 128],
                        wv_sb[:, kc, :],
                        start=(kc == 0),
                        stop=(kc == KC - 1),
                        skip=True,
                    )
                nc.vector.tensor_copy(Vt[:, 4 * c2 + s, :], ph)

            def oproj_unit(st, n):
                # full-bank unit; scheduled at most one per filler position
                # so the previous unit's DVE evacuation lands in time
                for k2 in range(2):
                    mm(
                        psw[:],
                        OTn[k2][:, st * 128 : (st + 1) * 128],
                        wo_sb[:, k2, n * 512 : (n + 1) * 512],
                        start=(k2 == 0),
                        stop=(k2 == 1),
                        skip=True,
                    )
                osb = osp.tile([128, 512], mmdt, tag="osb")
                nc.vector.tensor_copy(osb[:], psw[:])
                nc.sync.dma_start(
                    outd[st * 128 : (st + 1) * 128, n * 512 : (n + 1) * 512], osb[:]
                )

            def oproj_tail(st, n, pool, evac_act):
                ps = pool.tile([128, 512], f32, tag=pool_tag(pool), name=f"op{st}_{n}")
                for k2 in range(2):
                    mm(
                        ps[:],
                        OTn[k2][:, st * 128 : (st + 1) * 128],
                        wo_sb[:, k2, n * 512 : (n + 1) * 512],
                        start=(k2 == 0),
                        stop=(k2 == 1),
                        skip=True,
                    )
                osb = osp.tile([128, 512], mmdt, tag="osb")
                if evac_act:
                    nc.scalar.copy(osb[:], ps[:])
                else:
                    nc.vector.tensor_copy(osb[:], ps[:])
                nc.sync.dma_start(
                    outd[st * 128 : (st + 1) * 128, n * 512 : (n + 1) * 512], osb[:]
                )

            def pool_tag(pool):
                return {id(psS): "pss", id(psPO): "po", id(psSUM): "sums", id(psW): "psw"}[id(pool)]

            def bc_unit(c2, t, otrs, rr):
                for h2 in range(2):
                    head = 2 * t + h2
                    mm(
                        psw[h2 * 64 : h2 * 64 + 64, :],
                        ones_sb[32 * head : 32 * head + 1, :],
                        rr[32 * head : 32 * head + 1, :],
                        start=True,
                        stop=True,
                        tile_position=(32 * head, h2 * 64),
                        skip=True,
                    )
                nc.vector.tensor_mul(
                    OTn[t][:, c2 * SC : (c2 + 1) * SC], otrs[t][:], psw[:]
                )

            # ---- attention emission helpers, split per head-pair t so the
            # emission order matches the exp-stream release order ----
            def emit_scores_t(c, j, t):
                d = j - 4 * c
                x0 = max(0, 128 * d)
                ps = psS.tile([128, 1024], f32, tag="pss")
                for h2 in range(2):
                    p0 = h2 * 64
                    mm(
                        ps[:, h2 * 512 + x0 : (h2 + 1) * 512],
                        KT[t][p0 : p0 + 64, j * 128 : (j + 1) * 128],
                        QT[t][p0 : p0 + 64, c * SC + x0 : (c + 1) * SC],
                        start=True,
                        stop=True,
                    )
                pt = ptp.tile([128, 1024], mmdt, tag="pt")
                psv = ps.rearrange("p (h x) -> p h x", x=512)
                ptv = pt.rearrange("p (h x) -> p h x", x=512)
                nc.scalar.activation(ptv[:, :, x0:], psv[:, :, x0:], Exp)
                if d >= 0:
                    nc.vector.tensor_mul(
                        ptv[:, :, x0 : x0 + 128],
                        ptv[:, :, x0 : x0 + 128],
                        mask_sb[:, 0:1, :].broadcast_to([128, 2, 128]),
                    )
                return pt

            def emit_pv_t(c, j, jmax, pt, po, t):
                d = j - 4 * c
                x0 = max(0, 128 * d)
                for h2 in range(2):
                    h = 2 * t + h2
                    mm(
                        po[t][h2 * 64 : h2 * 64 + 64, x0:],
                        Vt[:, j, h * 64 : (h + 1) * 64],
                        pt[:, h2 * 512 + x0 : (h2 + 1) * 512],
                        start=(j == 0),
                        stop=(j == jmax),
                        skip=True,
                    )

            def emit_sums(c, j, jmax, pts, sums):
                d = j - 4 * c
                x0 = max(0, 128 * d)
                for h in range(HL):
                    t, h2 = divmod(h, 2)
                    mm(
                        sums[32 * h : 32 * h + 32, x0:],
                        ones_sb[:, 0:32],
                        pts[t][:, h2 * 512 + x0 : (h2 + 1) * 512],
                        start=(j == 0),
                        stop=(j == jmax),
                        tile_position=(0, 32 * h),
                        skip=True,
                    )

            def chunk_end(c, po, sums, last=False):
                def recip_chain():
                    rf = rsp.tile([128, 512], f32, tag="rf", name=f"rf{c}")
                    nc.vector.reciprocal_approx_fast(rf[:], sums[:])
                    rr = rsp.tile([128, 512], mmdt, tag="rr", name=f"rr{c}")
                    nc.vector.tensor_copy(rr[:], rf[:])
                    return rr

                def po_evac():
                    otrs = []
                    for t in range(2):
                        otr = orp.tile([128, 512], f32, tag="otr", name=f"otr{c}_{t}")
                        nc.vector.tensor_copy(otr[:], po[t][:])
                        otrs.append(otr)
                    return otrs

                if last:
                    rr = recip_chain()
                    otrs = po_evac()
                else:
                    otrs = po_evac()
                    rr = recip_chain()
                return otrs, rr

            # ---- per-chunk filler unit lists ----
            def qk_units(c2):
                xq, xk, _ = xts_all[c2]
                us = [
                    (lambda t=t, h=h, xq=xq, c2=c2: proj_qk_unit(c2, xq, wq_sb, bq_sb, QT, t, h))
                    for t in range(2) for h in range(2)
                ] + [
                    (lambda t=t, h=h, xk=xk, c2=c2: proj_qk_unit(c2, xk, wk_sb, bk_sb, KT, t, h))
                    for t in range(2) for h in range(2)
                ]
                return us

            def v_units(c2):
                xv = xts_all[c2][2]
                return [
                    (lambda s=s, xv=xv, c2=c2: proj_v_unit(c2, xv, s)) for s in range(4)
                ]

            def bc_units(c2, ep):
                otrs, rr = ep
                return [
                    (lambda t=t: bc_unit(c2, t, otrs, rr)) for t in range(2)
                ]

            def oproj_units(c2):
                return [
                    (lambda st=st, n=n: oproj_unit(st, n))
                    for st in range(4 * c2, 4 * c2 + 4) for n in range(2)
                ]

            # ---- main fused loop ----
            # Per-iteration emission order matches the release order of the
            # exp stream: scores(j+1,t) frees up as soon as exp(j,t) has read
            # its PSUM tile, and pv(j,t) as soon as exp(j,t)+mask are done,
            # so [S(t0), F, P(t0), S(t1), F, P(t1), SUMS] keeps the PE FIFO
            # head unblocked with filler units absorbing the exp latency.
            ep_states = {}
            pts_cur = None
            for c in range(NSC):
                jmax = 4 * c + 3
                nj = jmax + 1
                units = list(v_units(c))
                if c >= 1:
                    units += bc_units(c - 1, ep_states[c - 1])
                if c == 2:
                    units += oproj_units(0)
                if c == 3:
                    units += oproj_units(1) + oproj_units(2)
                if c + 1 < NSC:
                    units += qk_units(c + 1)

                po = [
                    psPO.tile([128, 512], f32, tag="po", name=f"po{c}_{t}")
                    for t in range(2)
                ]
                sums = psSUM.tile([128, 512], f32, tag="sums", name=f"sums{c}")

                pace = len(units) / nj
                acc = 0.0
                popped = 0
                if pts_cur is None:
                    pts_cur = [emit_scores_t(c, 0, 0), emit_scores_t(c, 0, 1)]
                for j in range(nj):
                    acc += pace
                    npop = int(acc)
                    acc -= npop
                    # V(c) units head the list; slot s=j-4c must be written
                    # before this iteration's pv reads Vt[:, j].
                    d = j - 4 * c
                    while d >= 0 and popped <= d and units:
                        units.pop(0)()
                        popped += 1
                        npop = max(0, npop - 1)
                    flush = j == nj - 1
                    if flush:
                        # All remaining units must precede the next chunk's
                        # scores: Tile dependencies follow program order, and
                        # the Q(c+1) units write the QT region that
                        # scores(c+1, 0) reads.
                        while units:
                            units.pop(0)()
                            popped += 1
                    elif npop >= 1 and units:
                        units.pop(0)()
                        popped += 1
                        npop -= 1
                    nxt = (c, j + 1) if j < jmax else (
                        (c + 1, 0) if c + 1 < NSC else None
                    )
                    pts_new = [None, None]
                    if nxt:
                        pts_new[0] = emit_scores_t(nxt[0], nxt[1], 0)
                        pts_new[1] = emit_scores_t(nxt[0], nxt[1], 1)
                    while not flush and npop > 0 and units:
                        units.pop(0)()
                        popped += 1
                        npop -= 1
                    emit_pv_t(c, j, jmax, pts_cur[0], po, 0)
                    emit_pv_t(c, j, jmax, pts_cur[1], po, 1)
                    emit_sums(c, j, jmax, pts_cur, sums)
                    pts_cur = pts_new
                ep_states[c] = chunk_end(c, po, sums, last=(c == NSC - 1))

            # ---- tail: last chunk's normalize + out-projection ----
            otrs3, rr3 = ep_states[NSC - 1]
            for t in range(2):
                bc_unit(NSC - 1, t, otrs3, rr3)
            tail_pools = [psPO, psSUM, psPO, psS]
            i = 0
            for st in range(4 * (NSC - 1), 4 * (NSC - 1) + 4):
                for n in range(2):
                    oproj_tail(st, n, tail_pools[i % 4], evac_act=(i % 2 == 0))
                    i += 1

            if DEBUG_TAPS:
                for t in range(2):
                    nc.sync.dma_start(dbg_qt[t], QT[t][:])
                    nc.sync.dma_start(dbg_kt[t], KT[t][:])
                    nc.sync.dma_start(dbg_otn[t], OTn[t][:])
                nc.sync.dma_start(dbg_vt[:], Vt[:])

    nc.compile()
    return nc


def _get_nc():
    key = ("nc", MM_DTYPE)
    if key not in _CACHE:
        _CACHE[key] = _build()
    return _CACHE[key]


def _warr(wT, n):
    """[K, n] -> [128, K//128, n] so the device DMA is contiguous."""
    K = wT.shape[0]
    return np.ascontiguousarray(wT.reshape(K // 128, 128, n).transpose(1, 0, 2))


def make_in_maps(q, k, v, Wq, bq, Wk, bk, Wv, bv, Wo, bo):
    """Host-side shard prep: per-core input dict."""
    f32 = np.float32
    md = {"f16": np.float16, "f32r": f32, "f32": f32}[MM_DTYPE]
    mask = (np.arange(128)[None, :] >= np.arange(128)[:, None]).astype(md)
    # per-batch transposes shared by the 4 cores of each batch.
    # chunk 0 of q/k is [KC, 128, SC] (kc-major: single-kc DMA pieces are
    # linear dram reads); chunks 1..3 and all of v are [c, 128, KC, SC]
    # (dense per chunk: whole-chunk DMAs are linear).
    def _x0arr(x):
        a = x.T.astype(md).reshape(KC, 128, NSC, SC)
        return np.ascontiguousarray(a[:, :, 0, :])

    def _xRarr(x, c0):
        a = x.T.astype(md).reshape(KC, 128, NSC, SC).transpose(2, 1, 0, 3)
        return np.ascontiguousarray(a[c0:])

    xq0T = [_x0arr(q[b]) for b in range(2)]
    xk0T = [_x0arr(k[b]) for b in range(2)]
    xqRT = [_xRarr(q[b], 1) for b in range(2)]
    xkRT = [_xRarr(k[b], 1) for b in range(2)]
    xvT = [_xRarr(v[b], 0) for b in range(2)]
    in_maps = []
    for c in range(8):
        b, g = c // 4, c % 4
        sl = slice(DL * g, DL * (g + 1))
        in_maps.append(
            {
                "xq0T": xq0T[b],
                "xk0T": xk0T[b],
                "xqRT": xqRT[b],
                "xkRT": xkRT[b],
                "xvT": xvT[b],
                "wqT": _warr((Wq[sl, :].T * f32(0.125)).astype(md), DL),
                "wkT": _warr(Wk[sl, :].T.astype(md), DL),
                "wvT": _warr(Wv[sl, :].T.astype(md), DL),
                "woT": _warr(Wo[:, sl].T.astype(md), D),
                "bqd": np.ascontiguousarray((bq[sl] * f32(0.125)).reshape(2, 128).T),
                "bkd": np.ascontiguousarray(bk[sl].reshape(2, 128).T),
                "maskd": mask,
            }
        )
    return in_maps


def kernel(q, k, v, Wq, bq, Wk, bk, Wv, bv, Wo, bo):
    from concourse.bass_utils import run_bass_kernel_spmd

    args = [np.asarray(a, dtype=np.float32) for a in (q, k, v, Wq, bq, Wk, bk, Wv, bv, Wo, bo)]
    q, k, v, Wq, bq, Wk, bk, Wv, bv, Wo, bo = args
    nc = _get_nc()
    in_maps = make_in_maps(q, k, v, Wq, bq, Wk, bk, Wv, bv, Wo, bo)
    tmpdir = os.environ.get("BASS_KERNEL_TMPDIR") or None
    res = run_bass_kernel_spmd(nc, in_maps, list(range(8)), trace=TRACE, tmpdir=tmpdir)
    if TRACE and res.exec_time_ns is not None:
        print(f"HW exec time: {res.exec_time_ns} ns")
        print(f"HW exec time mean: {res.mean_exec_time_ns} ns")
    out = np.zeros((2, S, D), np.float32)
    for c in range(8):
        out[c // 4] += res.results[c]["out"].astype(np.float32)
    out += (bv @ Wo.T + bo)[None, None, :]
    return out


# revision 29
# speedup vs baseline: 1.0547x; 1.0070x over previous
"""Multi-head causal self-attention (B=2, S=2048, D=1024, H=16) on 8 TRN2 cores.

Sharding: core c handles batch b = c//4 and head group g = c%4 (4 heads,
256 output dims). W_q/W_k/W_v are split column-wise per head group, W_o
row-wise; each core computes a partial [S, D] output product which the host
sums per batch (plus the (bv @ Wo.T + bo) row, exact because softmax rows
sum to 1).

v3 pipeline (PE-busy-driven rework of v2; baseline trace showed PE busy
133us of a 162us wall, with ~26us of exposed drains and ~19us of ACT time
spent on projection evacuations that blocked the PE filler chains):
  - ACT runs *only* the softmax exp stream (plus a table preload at t=0).
    All PSUM evacuations (Q/K bias-add, V copy, out-proj copy) moved to DVE
    (tensor_scalar_add with a [128,1] bias AP handles the bias broadcast).
  - Filler work (projections of later chunks, out-projection + normalize of
    earlier chunks) is emitted as fine-grained units paced fractionally
    across the attention j-loop, with per-chunk lists sized so the PE never
    starves while ACT grinds exp: c0 <- [V0,Q1,K1], c1 <- [V1,bc0,Q2,K2],
    c2 <- [V2,bc1,oproj0,Q3,K3], c3 <- [V3,bc2,oproj1,oproj2].
  - Q/K/V filler chains accumulate in 256-wide half-bank slices of one
    persistent PSUM tile, ping-ponging halves so a chain's DVE evacuation
    overlaps the next chain's matmuls instead of stalling them.
  - Chunk-0 Q/K projections stream against kc-granular DMA pieces of the
    first x chunk (weights on the gpsimd DMA queue, x on the sync queue),
    so the first exp lands ~9us after start instead of ~18us.
  - Scores matmuls row-pack 2 heads (K=64 at row groups 0/64); PV col-packs
    2 heads (M=64 at col groups 0/64); softmax denominators come from 4
    concurrent M=32 ones-matmuls at col groups 0/32/64/96.
  - Tail out-projection rotates PSUM banks (psPO/psSUM/psW) so evacuations
    overlap the remaining matmuls.
"""

import os
import sys

import numpy as np

# concourse (Bass/Tile) normally comes from PYTHONPATH; fall back to the
# container's copy when run from a bare directory.
for _p in ("/root/.axon_site/_ro/trn_rl_repo", "/opt/trn_rl_repo"):
    if _p not in sys.path and os.path.isdir(_p):
        sys.path.append(_p)

S = 2048
D = 1024
HL = 4          # heads per core
DL = 256        # local head dims per core
SC = 512        # sq chunk width
NSC = S // SC   # 4 chunks
KC = D // 128   # 8 contraction chunks for the projections

MM_DTYPE = os.environ.get("BASS_MM_DTYPE", "f16")
TRACE = os.environ.get("BASS_KERNEL_TRACE", "0") == "1"
DEBUG_TAPS = os.environ.get("BASS_KERNEL_DEBUG", "0") == "1"

_CACHE = {}


def _build():
    import concourse.bass as bass
    import concourse.mybir as mybir
    import concourse.tile as tile
    from concourse import bacc

    dt = mybir.dt
    f32 = dt.float32
    mmdt = {"f16": dt.float16, "f32r": dt.float32r, "f32": dt.float32}[MM_DTYPE]

    nc = bacc.Bacc("TRN2", target_bir_lowering=False, debug=False)

    # chunk 0 of q/k is kc-major so each single-kc piece DMA is a fully
    # linear dram read; later chunks (and all of v) are chunk-major dense
    # so each whole-chunk DMA is linear (4KB+ per partition, stride==run)
    xq0T = nc.dram_tensor("xq0T", [KC // 2, 128, 2, SC], mmdt, kind="ExternalInput").ap()
    xk0T = nc.dram_tensor("xk0T", [KC // 2, 128, 2, SC], mmdt, kind="ExternalInput").ap()
    xqRT = nc.dram_tensor("xqRT", [NSC - 1, 128, KC, SC], mmdt, kind="ExternalInput").ap()
    xkRT = nc.dram_tensor("xkRT", [NSC - 1, 128, KC, SC], mmdt, kind="ExternalInput").ap()
    xvT = nc.dram_tensor("xvT", [NSC, 128, KC, SC], mmdt, kind="ExternalInput").ap()
    wqT = nc.dram_tensor("wqT", [128, KC, DL], mmdt, kind="ExternalInput").ap()
    wkT = nc.dram_tensor("wkT", [128, KC, DL], mmdt, kind="ExternalInput").ap()
    wvT = nc.dram_tensor("wvT", [128, KC, DL], mmdt, kind="ExternalInput").ap()
    woT = nc.dram_tensor("woT", [128, 2, D], mmdt, kind="ExternalInput").ap()
    bqd = nc.dram_tensor("bqd", [128, 2], f32, kind="ExternalInput").ap()
    bkd = nc.dram_tensor("bkd", [128, 2], f32, kind="ExternalInput").ap()
    maskd = nc.dram_tensor("maskd", [128, 128], mmdt, kind="ExternalInput").ap()
    outd = nc.dram_tensor("out", [S, D], mmdt, kind="ExternalOutput").ap()
    if DEBUG_TAPS:
        dbg_qt = nc.dram_tensor("dbg_qt", [2, 128, S], mmdt, kind="ExternalOutput").ap()
        dbg_kt = nc.dram_tensor("dbg_kt", [2, 128, S], mmdt, kind="ExternalOutput").ap()
        dbg_vt = nc.dram_tensor("dbg_vt", [128, 16, 256], mmdt, kind="ExternalOutput").ap()
        dbg_otn = nc.dram_tensor("dbg_otn", [2, 128, S], mmdt, kind="ExternalOutput").ap()

    Exp = mybir.ActivationFunctionType.Exp

    def mm(ps, lhsT, rhs, start, stop, tile_position=None, skip=False):
        nc.tensor.matmul(
            ps, lhsT, rhs, start=start, stop=stop, tile_position=tile_position,
            skip_group_check=skip,
        )

    with tile.TileContext(nc) as tc:
        with (
            tc.tile_pool(name="const", bufs=1) as constp,
            tc.tile_pool(name="w", bufs=1) as wp,
            tc.tile_pool(name="x", bufs=10) as xp,
            tc.tile_pool(name="pp", bufs=1) as pp,
            tc.tile_pool(name="pt", bufs=4) as ptp,
            tc.tile_pool(name="otr", bufs=4) as orp,
            tc.tile_pool(name="rs", bufs=2) as rsp,
            tc.tile_pool(name="osb", bufs=4) as osp,
            tc.tile_pool(name="psS", bufs=2, space="PSUM") as psS,
            tc.tile_pool(name="psPO", bufs=2, space="PSUM") as psPO,
            tc.tile_pool(name="psSUM", bufs=1, space="PSUM") as psSUM,
            tc.tile_pool(name="psW", bufs=1, space="PSUM") as psW,
        ):
            # ---- DVE constants (first so warmup + exp preload can start) ----
            ones_f32 = constp.tile([128, 64], f32, tag="ones_f32")
            nc.vector.memset(ones_f32[:], 1.0)
            ones_sb = constp.tile([128, 64], mmdt, tag="ones")
            nc.vector.tensor_copy(ones_sb[:], ones_f32[:])
            wz = constp.tile([128, 512], mmdt, tag="wz")
            nc.vector.memset(wz[:], 0.0)

            # ---- DMA ordering (v5 scheme): weights on the scalar queue,
            # all x on the sync queue; chunk-0 q/k as single-kc linear
            # pieces so the preamble projections stream ----
            wq_sb = wp.tile([128, KC, DL], mmdt, tag="wq")
            nc.scalar.dma_start(wq_sb[:], wqT[:])
            bq_sb = constp.tile([128, 2], f32, tag="bq")
            nc.scalar.dma_start(bq_sb[:], bqd[:])
            wk_sb = wp.tile([128, KC, DL], mmdt, tag="wk")
            nc.scalar.dma_start(wk_sb[:], wkT[:])
            bk_sb = constp.tile([128, 2], f32, tag="bk")
            nc.scalar.dma_start(bk_sb[:], bkd[:])

            xq0 = xp.tile([128, KC, SC], mmdt, tag="x", name="xq0")
            for p in range(KC // 2):
                nc.sync.dma_start(xq0[:, 2 * p : 2 * p + 2, :], xq0T[p])
            xk0 = xp.tile([128, KC, SC], mmdt, tag="x", name="xk0")
            for p in range(KC // 2):
                nc.sync.dma_start(xk0[:, 2 * p : 2 * p + 2, :], xk0T[p])

            # exp table preload: after the critical scalar-queue DMA issues
            # (so it doesn't delay them) but well before the first real exp
            pre_sb = constp.tile([128, 8], mmdt, tag="pre")
            nc.scalar.activation(pre_sb[:], ones_f32[:, 0:8], Exp)

            mask_sb = constp.tile([128, 1, 128], mmdt, tag="mask")
            nc.scalar.dma_start(mask_sb[:, 0, :], maskd[:])
            wv_sb = wp.tile([128, KC, DL], mmdt, tag="wv")
            nc.scalar.dma_start(wv_sb[:], wvT[:])
            # xv0 on the scalar queue: the sync queue is busy with xq1/xk1
            # by the time V0 filler units need it
            xv0 = xp.tile([128, KC, SC], mmdt, tag="x", name="xv0")
            nc.scalar.dma_start(xv0[:], xvT[0])
            wo_sb = wp.tile([128, 2, D], mmdt, tag="wo")
            nc.scalar.dma_start(wo_sb[:], woT[:])

            xts_all = {0: [xq0, xk0, xv0]}
            srcs = {"q": xqRT, "k": xkRT}
            for cc in range(1, NSC):
                xts = []
                for nm in ("q", "k", "v"):
                    t = xp.tile([128, KC, SC], mmdt, tag="x", name=f"x{nm}{cc}")
                    src = xvT[cc] if nm == "v" else srcs[nm][cc - 1]
                    nc.sync.dma_start(t[:], src)
                    xts.append(t)
                xts_all[cc] = xts

            # ---- persistent PSUM accumulator for filler chains + warmup ----
            psw = psW.tile([128, 512], f32, tag="psw", name="psw")
            for _ in range(8):
                mm(psw[:], wz[:, 0:128], wz[:], start=True, stop=True, skip=True)

            # ---- persistent activations ----
            QT = [pp.tile([128, S], mmdt, tag=f"qt{t}", name=f"qt{t}") for t in range(2)]
            KT = [pp.tile([128, S], mmdt, tag=f"kt{t}", name=f"kt{t}") for t in range(2)]
            Vt = pp.tile([128, 16, 64 * HL], mmdt, tag="vt")
            OTn = [pp.tile([128, S], mmdt, tag=f"otn{t}", name=f"otn{t}") for t in range(2)]

            # ---- chunk-0 Q/K projections: kc-major streaming into psS ----
            def preamble_proj(xt, w_sb, b_sb, dstT):
                # One accumulation group per PSUM bank (group = t): start
                # pending-zeroes the whole 2KB zero region, so groups must
                # not interleave within a bank. Within a group the two
                # 256-wide h-subregions may interleave freely.
                ps = psS.tile([128, 1024], f32, tag="pss", name="pre_proj")
                for kc in range(KC):
                    for t in range(2):
                        for h in range(2):
                            mm(
                                ps[:, t * 512 + h * 256 : t * 512 + h * 256 + 256],
                                w_sb[:, kc, t * 128 : (t + 1) * 128],
                                xt[:, kc, h * 256 : h * 256 + 256],
                                start=(kc == 0 and h == 0),
                                stop=(kc == KC - 1 and h == 1),
                                skip=True,
                            )
                for t in range(2):
                    for h in range(2):
                        nc.vector.tensor_scalar_add(
                            dstT[t][:, h * 256 : h * 256 + 256],
                            ps[:, t * 512 + h * 256 : t * 512 + h * 256 + 256],
                            b_sb[:, t : t + 1],
                        )

            preamble_proj(xq0, wq_sb, bq_sb, QT)
            preamble_proj(xk0, wk_sb, bk_sb, KT)

            # ---- filler units (each: a few MMs + a DVE evacuation) ----
            whalf = [0]

            def take_half():
                h = whalf[0]
                whalf[0] ^= 1
                return psw[:, h * 256 : h * 256 + 256]

            def proj_qk_unit(c2, xt, w_sb, b_sb, dstT, t, h):
                ph = take_half()
                for kc in range(KC):
                    mm(
                        ph,
                        w_sb[:, kc, t * 128 : (t + 1) * 128],
                        xt[:, kc, h * 256 : h * 256 + 256],
                        start=(kc == 0),
                        stop=(kc == KC - 1),
                        skip=True,
                    )
                nc.vector.tensor_scalar_add(
                    dstT[t][:, c2 * SC + h * 256 : c2 * SC + h * 256 + 256],
                    ph,
                    b_sb[:, t : t + 1],
                )

            def proj_v_unit(c2, xt, s):
                ph = take_half()
                for kc in range(KC):
                    mm(
                        ph,
                        xt[:, kc, s * 128 : (s + 1) * 128],
                        wv_sb[:, kc, :],
                        start=(kc == 0),
                        stop=(kc == KC - 1),
                        skip=True,
                    )
                nc.vector.tensor_copy(Vt[:, 4 * c2 + s, :], ph)

            def oproj_unit(st, n):
                # full-bank unit; scheduled at most one per filler position
                # so the previous unit's DVE evacuation lands in time
                for k2 in range(2):
                    mm(
                        psw[:],
                        OTn[k2][:, st * 128 : (st + 1) * 128],
                        wo_sb[:, k2, n * 512 : (n + 1) * 512],
                        start=(k2 == 0),
                        stop=(k2 == 1),
                        skip=True,
                    )
                osb = osp.tile([128, 512], mmdt, tag="osb")
                nc.vector.tensor_copy(osb[:], psw[:])
                nc.sync.dma_start(
                    outd[st * 128 : (st + 1) * 128, n * 512 : (n + 1) * 512], osb[:]
                )

            def oproj_tail(st, n, pool, evac_act):
                ps = pool.tile([128, 512], f32, tag=pool_tag(pool), name=f"op{st}_{n}")
                for k2 in range(2):
                    mm(
                        ps[:],
                        OTn[k2][:, st * 128 : (st + 1) * 128],
                        wo_sb[:, k2, n * 512 : (n + 1) * 512],
                        start=(k2 == 0),
                        stop=(k2 == 1),
                        skip=True,
                    )
                osb = osp.tile([128, 512], mmdt, tag="osb")
                if evac_act:
                    nc.scalar.copy(osb[:], ps[:])
                else:
                    nc.vector.tensor_copy(osb[:], ps[:])
                nc.sync.dma_start(
                    outd[st * 128 : (st + 1) * 128, n * 512 : (n + 1) * 512], osb[:]
                )

            def pool_tag(pool):
                return {id(psS): "pss", id(psPO): "po", id(psSUM): "sums", id(psW): "psw"}[id(pool)]

            def bc_unit(c2, t, otrs, rr):
                for h2 in range(2):
                    head = 2 * t + h2
                    mm(
                        psw[h2 * 64 : h2 * 64 + 64, :],
                        ones_sb[32 * head : 32 * head + 1, :],
                        rr[32 * head : 32 * head + 1, :],
                        start=True,
                        stop=True,
                        tile_position=(32 * head, h2 * 64),
                        skip=True,
                    )
                nc.vector.tensor_mul(
                    OTn[t][:, c2 * SC : (c2 + 1) * SC], otrs[t][:], psw[:]
                )

            # ---- attention emission helpers, split per head-pair t so the
            # emission order matches the exp-stream release order ----
            def emit_scores_t(c, j, t):
                d = j - 4 * c
                x0 = max(0, 128 * d)
                ps = psS.tile([128, 1024], f32, tag="pss")
                for h2 in range(2):
                    p0 = h2 * 64
                    mm(
                        ps[:, h2 * 512 + x0 : (h2 + 1) * 512],
                        KT[t][p0 : p0 + 64, j * 128 : (j + 1) * 128],
                        QT[t][p0 : p0 + 64, c * SC + x0 : (c + 1) * SC],
                        start=True,
                        stop=True,
                    )
                pt = ptp.tile([128, 1024], mmdt, tag="pt")
                psv = ps.rearrange("p (h x) -> p h x", x=512)
                ptv = pt.rearrange("p (h x) -> p h x", x=512)
                nc.scalar.activation(ptv[:, :, x0:], psv[:, :, x0:], Exp)
                if d >= 0:
                    nc.vector.tensor_mul(
                        ptv[:, :, x0 : x0 + 128],
                        ptv[:, :, x0 : x0 + 128],
                        mask_sb[:, 0:1, :].broadcast_to([128, 2, 128]),
                    )
                return pt

            def emit_pv_t(c, j, jmax, pt, po, t):
                d = j - 4 * c
                x0 = max(0, 128 * d)
                for h2 in range(2):
                    h = 2 * t + h2
                    mm(
                        po[t][h2 * 64 : h2 * 64 + 64, x0:],
                        Vt[:, j, h * 64 : (h + 1) * 64],
                        pt[:, h2 * 512 + x0 : (h2 + 1) * 512],
                        start=(j == 0),
                        stop=(j == jmax),
                        skip=True,
                    )

            def emit_sums(c, j, jmax, pts, sums):
                d = j - 4 * c
                x0 = max(0, 128 * d)
                for h in range(HL):
                    t, h2 = divmod(h, 2)
                    mm(
                        sums[32 * h : 32 * h + 32, x0:],
                        ones_sb[:, 0:32],
                        pts[t][:, h2 * 512 + x0 : (h2 + 1) * 512],
                        start=(j == 0),
                        stop=(j == jmax),
                        tile_position=(0, 32 * h),
                        skip=True,
                    )

            def chunk_end(c, po, sums, last=False):
                def recip_chain():
                    rf = rsp.tile([128, 512], f32, tag="rf", name=f"rf{c}")
                    nc.vector.reciprocal_approx_fast(rf[:], sums[:])
                    rr = rsp.tile([128, 512], mmdt, tag="rr", name=f"rr{c}")
                    nc.vector.tensor_copy(rr[:], rf[:])
                    return rr

                def po_evac():
                    otrs = []
                    for t in range(2):
                        otr = orp.tile([128, 512], f32, tag="otr", name=f"otr{c}_{t}")
                        nc.vector.tensor_copy(otr[:], po[t][:])
                        otrs.append(otr)
                    return otrs

                if last:
                    rr = recip_chain()
                    otrs = po_evac()
                else:
                    otrs = po_evac()
                    rr = recip_chain()
                return otrs, rr

            # ---- per-chunk filler unit lists ----
            def qk_units(c2):
                xq, xk, _ = xts_all[c2]
                us = [
                    (lambda t=t, h=h, xq=xq, c2=c2: proj_qk_unit(c2, xq, wq_sb, bq_sb, QT, t, h))
                    for t in range(2) for h in range(2)
                ] + [
                    (lambda t=t, h=h, xk=xk, c2=c2: proj_qk_unit(c2, xk, wk_sb, bk_sb, KT, t, h))
                    for t in range(2) for h in range(2)
                ]
                return us

            def v_units(c2):
                xv = xts_all[c2][2]
                return [
                    (lambda s=s, xv=xv, c2=c2: proj_v_unit(c2, xv, s)) for s in range(4)
                ]

            def bc_units(c2, ep):
                otrs, rr = ep
                return [
                    (lambda t=t: bc_unit(c2, t, otrs, rr)) for t in range(2)
                ]

            def oproj_units(c2):
                return [
                    (lambda st=st, n=n: oproj_unit(st, n))
                    for st in range(4 * c2, 4 * c2 + 4) for n in range(2)
                ]

            # ---- main fused loop ----
            # Per-iteration emission order matches the release order of the
            # exp stream: scores(j+1,t) frees up as soon as exp(j,t) has read
            # its PSUM tile, and pv(j,t) as soon as exp(j,t)+mask are done,
            # so [S(t0), F, P(t0), S(t1), F, P(t1), SUMS] keeps the PE FIFO
            # head unblocked with filler units absorbing the exp latency.
            ep_states = {}
            pts_cur = None
            for c in range(NSC):
                jmax = 4 * c + 3
                nj = jmax + 1
                units = list(v_units(c))
                if c >= 1:
                    units += bc_units(c - 1, ep_states[c - 1])
                if c == 2:
                    units += oproj_units(0)
                if c == 3:
                    units += oproj_units(1) + oproj_units(2)
                if c + 1 < NSC:
                    units += qk_units(c + 1)

                po = [
                    psPO.tile([128, 512], f32, tag="po", name=f"po{c}_{t}")
                    for t in range(2)
                ]
                sums = psSUM.tile([128, 512], f32, tag="sums", name=f"sums{c}")

                pace = len(units) / nj
                acc = 0.0
                popped = 0
                if pts_cur is None:
                    pts_cur = [emit_scores_t(c, 0, 0), emit_scores_t(c, 0, 1)]
                for j in range(nj):
                    acc += pace
                    npop = int(acc)
                    acc -= npop
                    # V(c) units head the list; slot s=j-4c must be written
                    # before this iteration's pv reads Vt[:, j].
                    d = j - 4 * c
                    while d >= 0 and popped <= d and units:
                        units.pop(0)()
                        popped += 1
                        npop = max(0, npop - 1)
                    flush = j == nj - 1
                    if flush:
                        # All remaining units must precede the next chunk's
                        # scores: Tile dependencies follow program order, and
                        # the Q(c+1) units write the QT region that
                        # scores(c+1, 0) reads.
                        while units:
                            units.pop(0)()
                            popped += 1
                    elif npop >= 1 and units:
                        units.pop(0)()
                        popped += 1
                        npop -= 1
                    nxt = (c, j + 1) if j < jmax else (
                        (c + 1, 0) if c + 1 < NSC else None
                    )
                    pts_new = [None, None]
                    if nxt:
                        pts_new[0] = emit_scores_t(nxt[0], nxt[1], 0)
                        pts_new[1] = emit_scores_t(nxt[0], nxt[1], 1)
                    while not flush and npop > 0 and units:
                        units.pop(0)()
                        popped += 1
                        npop -= 1
                    emit_pv_t(c, j, jmax, pts_cur[0], po, 0)
                    emit_pv_t(c, j, jmax, pts_cur[1], po, 1)
                    emit_sums(c, j, jmax, pts_cur, sums)
                    pts_cur = pts_new
                ep_states[c] = chunk_end(c, po, sums, last=(c == NSC - 1))

            # ---- tail: last chunk's normalize + out-projection ----
            otrs3, rr3 = ep_states[NSC - 1]
            for t in range(2):
                bc_unit(NSC - 1, t, otrs3, rr3)
            tail_pools = [psPO, psSUM, psPO, psS]
            i = 0
            for st in range(4 * (NSC - 1), 4 * (NSC - 1) + 4):
                for n in range(2):
                    oproj_tail(st, n, tail_pools[i % 4], evac_act=(i % 2 == 0))
                    i += 1

            if DEBUG_TAPS:
                for t in range(2):
                    nc.sync.dma_start(dbg_qt[t], QT[t][:])
                    nc.sync.dma_start(dbg_kt[t], KT[t][:])
                    nc.sync.dma_start(dbg_otn[t], OTn[t][:])
                nc.sync.dma_start(dbg_vt[:], Vt[:])

    nc.compile()
    return nc


def _get_nc():
    key = ("nc", MM_DTYPE)
    if key not in _CACHE:
        _CACHE[key] = _build()
    return _CACHE[key]


def _warr(wT, n):
    """[K, n] -> [128, K//128, n] so the device DMA is contiguous."""
    K = wT.shape[0]
    return np.ascontiguousarray(wT.reshape(K // 128, 128, n).transpose(1, 0, 2))


def make_in_maps(q, k, v, Wq, bq, Wk, bk, Wv, bv, Wo, bo):
    """Host-side shard prep: per-core input dict."""
    f32 = np.float32
    md = {"f16": np.float16, "f32r": f32, "f32": f32}[MM_DTYPE]
    mask = (np.arange(128)[None, :] >= np.arange(128)[:, None]).astype(md)
    # per-batch transposes shared by the 4 cores of each batch.
    # chunk 0 of q/k is [KC, 128, SC] (kc-major: single-kc DMA pieces are
    # linear dram reads); chunks 1..3 and all of v are [c, 128, KC, SC]
    # (dense per chunk: whole-chunk DMAs are linear).
    def _x0arr(x):
        a = x.T.astype(md).reshape(KC, 128, NSC, SC)
        b = a[:, :, 0, :].reshape(KC // 2, 2, 128, SC).transpose(0, 2, 1, 3)
        return np.ascontiguousarray(b)

    def _xRarr(x, c0):
        a = x.T.astype(md).reshape(KC, 128, NSC, SC).transpose(2, 1, 0, 3)
        return np.ascontiguousarray(a[c0:])

    xq0T = [_x0arr(q[b]) for b in range(2)]
    xk0T = [_x0arr(k[b]) for b in range(2)]
    xqRT = [_xRarr(q[b], 1) for b in range(2)]
    xkRT = [_xRarr(k[b], 1) for b in range(2)]
    xvT = [_xRarr(v[b], 0) for b in range(2)]
    in_maps = []
    for c in range(8):
        b, g = c // 4, c % 4
        sl = slice(DL * g, DL * (g + 1))
        in_maps.append(
            {
                "xq0T": xq0T[b],
                "xk0T": xk0T[b],
                "xqRT": xqRT[b],
                "xkRT": xkRT[b],
                "xvT": xvT[b],
                "wqT": _warr((Wq[sl, :].T * f32(0.125)).astype(md), DL),
                "wkT": _warr(Wk[sl, :].T.astype(md), DL),
                "wvT": _warr(Wv[sl, :].T.astype(md), DL),
                "woT": _warr(Wo[:, sl].T.astype(md), D),
                "bqd": np.ascontiguousarray((bq[sl] * f32(0.125)).reshape(2, 128).T),
                "bkd": np.ascontiguousarray(bk[sl].reshape(2, 128).T),
                "maskd": mask,
            }
        )
    return in_maps


def kernel(q, k, v, Wq, bq, Wk, bk, Wv, bv, Wo, bo):
    from concourse.bass_utils import run_bass_kernel_spmd

    args = [np.asarray(a, dtype=np.float32) for a in (q, k, v, Wq, bq, Wk, bk, Wv, bv, Wo, bo)]
    q, k, v, Wq, bq, Wk, bk, Wv, bv, Wo, bo = args
    nc = _get_nc()
    in_maps = make_in_maps(q, k, v, Wq, bq, Wk, bk, Wv, bv, Wo, bo)
    tmpdir = os.environ.get("BASS_KERNEL_TMPDIR") or None
    res = run_bass_kernel_spmd(nc, in_maps, list(range(8)), trace=TRACE, tmpdir=tmpdir)
    if TRACE and res.exec_time_ns is not None:
        print(f"HW exec time: {res.exec_time_ns} ns")
        print(f"HW exec time mean: {res.mean_exec_time_ns} ns")
    out = np.zeros((2, S, D), np.float32)
    for c in range(8):
        out[c // 4] += res.results[c]["out"].astype(np.float32)
    out += (bv @ Wo.T + bo)[None, None, :]
    return out


# revision 30
# speedup vs baseline: 1.0923x; 1.0356x over previous
"""Multi-head causal self-attention (B=2, S=2048, D=1024, H=16) on 8 TRN2 cores.

Sharding: core c handles batch b = c//4 and head group g = c%4 (4 heads,
256 output dims). W_q/W_k/W_v are split column-wise per head group, W_o
row-wise; each core computes a partial [S, D] output product which the host
sums per batch (plus the (bv @ Wo.T + bo) row, exact because softmax rows
sum to 1).

v3 pipeline (PE-busy-driven rework of v2; baseline trace showed PE busy
133us of a 162us wall, with ~26us of exposed drains and ~19us of ACT time
spent on projection evacuations that blocked the PE filler chains):
  - ACT runs *only* the softmax exp stream (plus a table preload at t=0).
    All PSUM evacuations (Q/K bias-add, V copy, out-proj copy) moved to DVE
    (tensor_scalar_add with a [128,1] bias AP handles the bias broadcast).
  - Filler work (projections of later chunks, out-projection + normalize of
    earlier chunks) is emitted as fine-grained units paced fractionally
    across the attention j-loop, with per-chunk lists sized so the PE never
    starves while ACT grinds exp: c0 <- [V0,Q1,K1], c1 <- [V1,bc0,Q2,K2],
    c2 <- [V2,bc1,oproj0,Q3,K3], c3 <- [V3,bc2,oproj1,oproj2].
  - Q/K/V filler chains accumulate in 256-wide half-bank slices of one
    persistent PSUM tile, ping-ponging halves so a chain's DVE evacuation
    overlaps the next chain's matmuls instead of stalling them.
  - Chunk-0 Q/K projections stream against kc-granular DMA pieces of the
    first x chunk (weights on the gpsimd DMA queue, x on the sync queue),
    so the first exp lands ~9us after start instead of ~18us.
  - Scores matmuls row-pack 2 heads (K=64 at row groups 0/64); PV col-packs
    2 heads (M=64 at col groups 0/64); softmax denominators come from 4
    concurrent M=32 ones-matmuls at col groups 0/32/64/96.
  - Tail out-projection rotates PSUM banks (psPO/psSUM/psW) so evacuations
    overlap the remaining matmuls.
"""

import os
import sys

import numpy as np

# concourse (Bass/Tile) normally comes from PYTHONPATH; fall back to the
# container's copy when run from a bare directory.
for _p in ("/root/.axon_site/_ro/trn_rl_repo", "/opt/trn_rl_repo"):
    if _p not in sys.path and os.path.isdir(_p):
        sys.path.append(_p)

S = 2048
D = 1024
HL = 4          # heads per core
DL = 256        # local head dims per core
SC = 512        # sq chunk width
NSC = S // SC   # 4 chunks
KC = D // 128   # 8 contraction chunks for the projections

MM_DTYPE = os.environ.get("BASS_MM_DTYPE", "f16")
TRACE = os.environ.get("BASS_KERNEL_TRACE", "0") == "1"
DEBUG_TAPS = os.environ.get("BASS_KERNEL_DEBUG", "0") == "1"

_CACHE = {}


def _build():
    import concourse.bass as bass
    import concourse.mybir as mybir
    import concourse.tile as tile
    from concourse import bacc

    dt = mybir.dt
    f32 = dt.float32
    mmdt = {"f16": dt.float16, "f32r": dt.float32r, "f32": dt.float32}[MM_DTYPE]

    nc = bacc.Bacc("TRN2", target_bir_lowering=False, debug=False)

    # chunk 0 of q/k is kc-major so each single-kc piece DMA is a fully
    # linear dram read; later chunks (and all of v) are chunk-major dense
    # so each whole-chunk DMA is linear (4KB+ per partition, stride==run)
    xq0T = nc.dram_tensor("xq0T", [KC // 2, 128, 2, SC], mmdt, kind="ExternalInput").ap()
    xk0T = nc.dram_tensor("xk0T", [KC // 2, 128, 2, SC], mmdt, kind="ExternalInput").ap()
    xqRT = nc.dram_tensor("xqRT", [NSC - 1, 128, KC, SC], mmdt, kind="ExternalInput").ap()
    xkRT = nc.dram_tensor("xkRT", [NSC - 1, 128, KC, SC], mmdt, kind="ExternalInput").ap()
    xvT = nc.dram_tensor("xvT", [NSC, 128, KC, SC], mmdt, kind="ExternalInput").ap()
    wqT = nc.dram_tensor("wqT", [128, KC, DL], mmdt, kind="ExternalInput").ap()
    wkT = nc.dram_tensor("wkT", [128, KC, DL], mmdt, kind="ExternalInput").ap()
    wvT = nc.dram_tensor("wvT", [128, KC, DL], mmdt, kind="ExternalInput").ap()
    woT = nc.dram_tensor("woT", [128, 2, D], mmdt, kind="ExternalInput").ap()
    bqd = nc.dram_tensor("bqd", [128, 2], f32, kind="ExternalInput").ap()
    bkd = nc.dram_tensor("bkd", [128, 2], f32, kind="ExternalInput").ap()
    maskd = nc.dram_tensor("maskd", [128, 128], mmdt, kind="ExternalInput").ap()
    outd = nc.dram_tensor("out", [S, D], mmdt, kind="ExternalOutput").ap()
    if DEBUG_TAPS:
        dbg_qt = nc.dram_tensor("dbg_qt", [2, 128, S], mmdt, kind="ExternalOutput").ap()
        dbg_kt = nc.dram_tensor("dbg_kt", [2, 128, S], mmdt, kind="ExternalOutput").ap()
        dbg_vt = nc.dram_tensor("dbg_vt", [128, 16, 256], mmdt, kind="ExternalOutput").ap()
        dbg_otn = nc.dram_tensor("dbg_otn", [2, 128, S], mmdt, kind="ExternalOutput").ap()

    Exp = mybir.ActivationFunctionType.Exp

    def mm(ps, lhsT, rhs, start, stop, tile_position=None, skip=False):
        nc.tensor.matmul(
            ps, lhsT, rhs, start=start, stop=stop, tile_position=tile_position,
            skip_group_check=skip,
        )

    with tile.TileContext(nc) as tc:
        with (
            tc.tile_pool(name="const", bufs=1) as constp,
            tc.tile_pool(name="w", bufs=1) as wp,
            tc.tile_pool(name="x", bufs=10) as xp,
            tc.tile_pool(name="pp", bufs=1) as pp,
            tc.tile_pool(name="pt", bufs=4) as ptp,
            tc.tile_pool(name="otr", bufs=4) as orp,
            tc.tile_pool(name="rs", bufs=2) as rsp,
            tc.tile_pool(name="osb", bufs=4) as osp,
            tc.tile_pool(name="psS", bufs=2, space="PSUM") as psS,
            tc.tile_pool(name="psPO", bufs=2, space="PSUM") as psPO,
            tc.tile_pool(name="psSUM", bufs=1, space="PSUM") as psSUM,
            tc.tile_pool(name="psW", bufs=1, space="PSUM") as psW,
        ):
            # ---- DVE constants (first so warmup + exp preload can start) ----
            ones_f32 = constp.tile([128, 64], f32, tag="ones_f32")
            nc.vector.memset(ones_f32[:], 1.0)
            ones_sb = constp.tile([128, 64], mmdt, tag="ones")
            nc.vector.tensor_copy(ones_sb[:], ones_f32[:])
            wz = constp.tile([128, 512], mmdt, tag="wz")
            nc.vector.memset(wz[:], 0.0)

            # ---- DMA ordering (v5 scheme): weights on the scalar queue,
            # all x on the sync queue; chunk-0 q/k as single-kc linear
            # pieces so the preamble projections stream ----
            wq_sb = wp.tile([128, KC, DL], mmdt, tag="wq")
            nc.scalar.dma_start(wq_sb[:], wqT[:])
            bq_sb = constp.tile([128, 2], f32, tag="bq")
            nc.scalar.dma_start(bq_sb[:], bqd[:])
            wk_sb = wp.tile([128, KC, DL], mmdt, tag="wk")
            nc.scalar.dma_start(wk_sb[:], wkT[:])
            bk_sb = constp.tile([128, 2], f32, tag="bk")
            nc.scalar.dma_start(bk_sb[:], bkd[:])

            xq0 = xp.tile([128, KC, SC], mmdt, tag="x", name="xq0")
            for p in range(KC // 2):
                nc.sync.dma_start(xq0[:, 2 * p : 2 * p + 2, :], xq0T[p])
            xk0 = xp.tile([128, KC, SC], mmdt, tag="x", name="xk0")
            for p in range(KC // 2):
                nc.sync.dma_start(xk0[:, 2 * p : 2 * p + 2, :], xk0T[p])

            # exp table preload: after the critical scalar-queue DMA issues
            # (so it doesn't delay them) but well before the first real exp
            pre_sb = constp.tile([128, 8], mmdt, tag="pre")
            nc.scalar.activation(pre_sb[:], ones_f32[:, 0:8], Exp)

            wv_sb = wp.tile([128, KC, DL], mmdt, tag="wv")
            nc.scalar.dma_start(wv_sb[:], wvT[:])
            mask_sb = constp.tile([128, 1, 128], mmdt, tag="mask")
            nc.scalar.dma_start(mask_sb[:, 0, :], maskd[:])
            wo_sb = wp.tile([128, 2, D], mmdt, tag="wo")
            nc.scalar.dma_start(wo_sb[:], woT[:])

            xv0 = xp.tile([128, KC, SC], mmdt, tag="x", name="xv0")
            nc.sync.dma_start(xv0[:], xvT[0])
            xts_all = {0: [xq0, xk0, xv0]}
            srcs = {"q": xqRT, "k": xkRT}
            for cc in range(1, NSC):
                xts = []
                for nm in ("q", "k", "v"):
                    t = xp.tile([128, KC, SC], mmdt, tag="x", name=f"x{nm}{cc}")
                    src = xvT[cc] if nm == "v" else srcs[nm][cc - 1]
                    nc.sync.dma_start(t[:], src)
                    xts.append(t)
                xts_all[cc] = xts

            # ---- persistent PSUM accumulator for filler chains + warmup ----
            psw = psW.tile([128, 512], f32, tag="psw", name="psw")
            for _ in range(8):
                mm(psw[:], wz[:, 0:128], wz[:], start=True, stop=True, skip=True)

            # ---- persistent activations ----
            QT = [pp.tile([128, S], mmdt, tag=f"qt{t}", name=f"qt{t}") for t in range(2)]
            KT = [pp.tile([128, S], mmdt, tag=f"kt{t}", name=f"kt{t}") for t in range(2)]
            Vt = pp.tile([128, 16, 64 * HL], mmdt, tag="vt")
            OTn = [pp.tile([128, S], mmdt, tag=f"otn{t}", name=f"otn{t}") for t in range(2)]

            # ---- chunk-0 Q/K projections: kc-major streaming into psS ----
            def preamble_proj(xt, w_sb, b_sb, dstT):
                # One accumulation group per PSUM bank (group = t): start
                # pending-zeroes the whole 2KB zero region, so groups must
                # not interleave within a bank. Within a group the two
                # 256-wide h-subregions may interleave freely.
                ps = psS.tile([128, 1024], f32, tag="pss", name="pre_proj")
                for kc in range(KC):
                    for t in range(2):
                        for h in range(2):
                            mm(
                                ps[:, t * 512 + h * 256 : t * 512 + h * 256 + 256],
                                w_sb[:, kc, t * 128 : (t + 1) * 128],
                                xt[:, kc, h * 256 : h * 256 + 256],
                                start=(kc == 0 and h == 0),
                                stop=(kc == KC - 1 and h == 1),
                                skip=True,
                            )
                for t in range(2):
                    for h in range(2):
                        nc.vector.tensor_scalar_add(
                            dstT[t][:, h * 256 : h * 256 + 256],
                            ps[:, t * 512 + h * 256 : t * 512 + h * 256 + 256],
                            b_sb[:, t : t + 1],
                        )

            preamble_proj(xq0, wq_sb, bq_sb, QT)
            preamble_proj(xk0, wk_sb, bk_sb, KT)

            # ---- filler units (each: a few MMs + a DVE evacuation) ----
            whalf = [0]

            def take_half():
                h = whalf[0]
                whalf[0] ^= 1
                return psw[:, h * 256 : h * 256 + 256]

            def proj_qk_unit(c2, xt, w_sb, b_sb, dstT, t, h):
                ph = take_half()
                for kc in range(KC):
                    mm(
                        ph,
                        w_sb[:, kc, t * 128 : (t + 1) * 128],
                        xt[:, kc, h * 256 : h * 256 + 256],
                        start=(kc == 0),
                        stop=(kc == KC - 1),
                        skip=True,
                    )
                nc.vector.tensor_scalar_add(
                    dstT[t][:, c2 * SC + h * 256 : c2 * SC + h * 256 + 256],
                    ph,
                    b_sb[:, t : t + 1],
                )

            def proj_v_unit(c2, xt, s):
                ph = take_half()
                for kc in range(KC):
                    mm(
                        ph,
                        xt[:, kc, s * 128 : (s + 1) * 128],
                        wv_sb[:, kc, :],
                        start=(kc == 0),
                        stop=(kc == KC - 1),
                        skip=True,
                    )
                nc.vector.tensor_copy(Vt[:, 4 * c2 + s, :], ph)

            def oproj_unit(st, n):
                # full-bank unit; scheduled at most one per filler position
                # so the previous unit's DVE evacuation lands in time
                for k2 in range(2):
                    mm(
                        psw[:],
                        OTn[k2][:, st * 128 : (st + 1) * 128],
                        wo_sb[:, k2, n * 512 : (n + 1) * 512],
                        start=(k2 == 0),
                        stop=(k2 == 1),
                        skip=True,
                    )
                osb = osp.tile([128, 512], mmdt, tag="osb")
                nc.vector.tensor_copy(osb[:], psw[:])
                nc.sync.dma_start(
                    outd[st * 128 : (st + 1) * 128, n * 512 : (n + 1) * 512], osb[:]
                )

            def oproj_tail(st, n, pool, evac_act):
                ps = pool.tile([128, 512], f32, tag=pool_tag(pool), name=f"op{st}_{n}")
                for k2 in range(2):
                    mm(
                        ps[:],
                        OTn[k2][:, st * 128 : (st + 1) * 128],
                        wo_sb[:, k2, n * 512 : (n + 1) * 512],
                        start=(k2 == 0),
                        stop=(k2 == 1),
                        skip=True,
                    )
                osb = osp.tile([128, 512], mmdt, tag="osb")
                if evac_act:
                    nc.scalar.copy(osb[:], ps[:])
                else:
                    nc.vector.tensor_copy(osb[:], ps[:])
                nc.sync.dma_start(
                    outd[st * 128 : (st + 1) * 128, n * 512 : (n + 1) * 512], osb[:]
                )

            def pool_tag(pool):
                return {id(psS): "pss", id(psPO): "po", id(psSUM): "sums", id(psW): "psw"}[id(pool)]

            def bc_unit(c2, t, otrs, rr):
                for h2 in range(2):
                    head = 2 * t + h2
                    mm(
                        psw[h2 * 64 : h2 * 64 + 64, :],
                        ones_sb[32 * head : 32 * head + 1, :],
                        rr[32 * head : 32 * head + 1, :],
                        start=True,
                        stop=True,
                        tile_position=(32 * head, h2 * 64),
                        skip=True,
                    )
                nc.vector.tensor_mul(
                    OTn[t][:, c2 * SC : (c2 + 1) * SC], otrs[t][:], psw[:]
                )

            # ---- attention emission helpers, split per head-pair t so the
            # emission order matches the exp-stream release order ----
            def emit_scores_t(c, j, t):
                d = j - 4 * c
                x0 = max(0, 128 * d)
                ps = psS.tile([128, 1024], f32, tag="pss")
                for h2 in range(2):
                    p0 = h2 * 64
                    mm(
                        ps[:, h2 * 512 + x0 : (h2 + 1) * 512],
                        KT[t][p0 : p0 + 64, j * 128 : (j + 1) * 128],
                        QT[t][p0 : p0 + 64, c * SC + x0 : (c + 1) * SC],
                        start=True,
                        stop=True,
                    )
                pt = ptp.tile([128, 1024], mmdt, tag="pt")
                psv = ps.rearrange("p (h x) -> p h x", x=512)
                ptv = pt.rearrange("p (h x) -> p h x", x=512)
                nc.scalar.activation(ptv[:, :, x0:], psv[:, :, x0:], Exp)
                if d >= 0:
                    nc.vector.tensor_mul(
                        ptv[:, :, x0 : x0 + 128],
                        ptv[:, :, x0 : x0 + 128],
                        mask_sb[:, 0:1, :].broadcast_to([128, 2, 128]),
                    )
                return pt

            def emit_pv_t(c, j, jmax, pt, po, t):
                d = j - 4 * c
                x0 = max(0, 128 * d)
                for h2 in range(2):
                    h = 2 * t + h2
                    mm(
                        po[t][h2 * 64 : h2 * 64 + 64, x0:],
                        Vt[:, j, h * 64 : (h + 1) * 64],
                        pt[:, h2 * 512 + x0 : (h2 + 1) * 512],
                        start=(j == 0),
                        stop=(j == jmax),
                        skip=True,
                    )

            def emit_sums(c, j, jmax, pts, sums):
                d = j - 4 * c
                x0 = max(0, 128 * d)
                for h in range(HL):
                    t, h2 = divmod(h, 2)
                    mm(
                        sums[32 * h : 32 * h + 32, x0:],
                        ones_sb[:, 0:32],
                        pts[t][:, h2 * 512 + x0 : (h2 + 1) * 512],
                        start=(j == 0),
                        stop=(j == jmax),
                        tile_position=(0, 32 * h),
                        skip=True,
                    )

            def chunk_end(c, po, sums, last=False):
                def recip_chain():
                    rf = rsp.tile([128, 512], f32, tag="rf", name=f"rf{c}")
                    nc.vector.reciprocal_approx_fast(rf[:], sums[:])
                    rr = rsp.tile([128, 512], mmdt, tag="rr", name=f"rr{c}")
                    nc.vector.tensor_copy(rr[:], rf[:])
                    return rr

                def po_evac():
                    otrs = []
                    for t in range(2):
                        otr = orp.tile([128, 512], f32, tag="otr", name=f"otr{c}_{t}")
                        nc.vector.tensor_copy(otr[:], po[t][:])
                        otrs.append(otr)
                    return otrs

                if last:
                    rr = recip_chain()
                    otrs = po_evac()
                else:
                    otrs = po_evac()
                    rr = recip_chain()
                return otrs, rr

            # ---- per-chunk filler unit lists ----
            def qk_units(c2):
                xq, xk, _ = xts_all[c2]
                us = [
                    (lambda t=t, h=h, xq=xq, c2=c2: proj_qk_unit(c2, xq, wq_sb, bq_sb, QT, t, h))
                    for t in range(2) for h in range(2)
                ] + [
                    (lambda t=t, h=h, xk=xk, c2=c2: proj_qk_unit(c2, xk, wk_sb, bk_sb, KT, t, h))
                    for t in range(2) for h in range(2)
                ]
                return us

            def v_units(c2):
                xv = xts_all[c2][2]
                return [
                    (lambda s=s, xv=xv, c2=c2: proj_v_unit(c2, xv, s)) for s in range(4)
                ]

            def bc_units(c2, ep):
                otrs, rr = ep
                return [
                    (lambda t=t: bc_unit(c2, t, otrs, rr)) for t in range(2)
                ]

            def oproj_units(c2):
                return [
                    (lambda st=st, n=n: oproj_unit(st, n))
                    for st in range(4 * c2, 4 * c2 + 4) for n in range(2)
                ]

            # ---- main fused loop ----
            # Per-iteration emission order matches the release order of the
            # exp stream: scores(j+1,t) frees up as soon as exp(j,t) has read
            # its PSUM tile, and pv(j,t) as soon as exp(j,t)+mask are done,
            # so [S(t0), F, P(t0), S(t1), F, P(t1), SUMS] keeps the PE FIFO
            # head unblocked with filler units absorbing the exp latency.
            ep_states = {}
            pts_cur = None
            for c in range(NSC):
                jmax = 4 * c + 3
                nj = jmax + 1
                units = list(v_units(c))
                if c >= 1:
                    units += bc_units(c - 1, ep_states[c - 1])
                if c == 2:
                    units += oproj_units(0)
                if c == 3:
                    units += oproj_units(1) + oproj_units(2)
                if c + 1 < NSC:
                    units += qk_units(c + 1)

                po = [
                    psPO.tile([128, 512], f32, tag="po", name=f"po{c}_{t}")
                    for t in range(2)
                ]
                sums = psSUM.tile([128, 512], f32, tag="sums", name=f"sums{c}")

                pace = len(units) / nj
                acc = 0.0
                popped = 0
                if pts_cur is None:
                    pts_cur = [emit_scores_t(c, 0, 0), emit_scores_t(c, 0, 1)]
                for j in range(nj):
                    acc += pace
                    npop = int(acc)
                    acc -= npop
                    # V(c) units head the list; slot s=j-4c must be written
                    # before this iteration's pv reads Vt[:, j].
                    d = j - 4 * c
                    while d >= 0 and popped <= d and units:
                        units.pop(0)()
                        popped += 1
                        npop = max(0, npop - 1)
                    flush = j == nj - 1
                    if flush:
                        # All remaining units must precede the next chunk's
                        # scores: Tile dependencies follow program order, and
                        # the Q(c+1) units write the QT region that
                        # scores(c+1, 0) reads.
                        while units:
                            units.pop(0)()
                            popped += 1
                    elif npop >= 1 and units:
                        units.pop(0)()
                        popped += 1
                        npop -= 1
                    nxt = (c, j + 1) if j < jmax else (
                        (c + 1, 0) if c + 1 < NSC else None
                    )
                    pts_new = [None, None]
                    if nxt:
                        pts_new[0] = emit_scores_t(nxt[0], nxt[1], 0)
                        pts_new[1] = emit_scores_t(nxt[0], nxt[1], 1)
                    while not flush and npop > 0 and units:
                        units.pop(0)()
                        popped += 1
                        npop -= 1
                    emit_pv_t(c, j, jmax, pts_cur[0], po, 0)
                    emit_pv_t(c, j, jmax, pts_cur[1], po, 1)
                    emit_sums(c, j, jmax, pts_cur, sums)
                    pts_cur = pts_new
                ep_states[c] = chunk_end(c, po, sums, last=(c == NSC - 1))

            # ---- tail: last chunk's normalize + out-projection ----
            otrs3, rr3 = ep_states[NSC - 1]
            for t in range(2):
                bc_unit(NSC - 1, t, otrs3, rr3)
            tail_pools = [psPO, psSUM, psPO, psS]
            i = 0
            for st in range(4 * (NSC - 1), 4 * (NSC - 1) + 4):
                for n in range(2):
                    oproj_tail(st, n, tail_pools[i % 4], evac_act=(i % 2 == 0))
                    i += 1

            if DEBUG_TAPS:
                for t in range(2):
                    nc.sync.dma_start(dbg_qt[t], QT[t][:])
                    nc.sync.dma_start(dbg_kt[t], KT[t][:])
                    nc.sync.dma_start(dbg_otn[t], OTn[t][:])
                nc.sync.dma_start(dbg_vt[:], Vt[:])

    nc.compile()
    return nc


def _get_nc():
    key = ("nc", MM_DTYPE)
    if key not in _CACHE:
        _CACHE[key] = _build()
    return _CACHE[key]


def _warr(wT, n):
    """[K, n] -> [128, K//128, n] so the device DMA is contiguous."""
    K = wT.shape[0]
    return np.ascontiguousarray(wT.reshape(K // 128, 128, n).transpose(1, 0, 2))


def make_in_maps(q, k, v, Wq, bq, Wk, bk, Wv, bv, Wo, bo):
    """Host-side shard prep: per-core input dict."""
    f32 = np.float32
    md = {"f16": np.float16, "f32r": f32, "f32": f32}[MM_DTYPE]
    mask = (np.arange(128)[None, :] >= np.arange(128)[:, None]).astype(md)
    # per-batch transposes shared by the 4 cores of each batch.
    # chunk 0 of q/k is [KC, 128, SC] (kc-major: single-kc DMA pieces are
    # linear dram reads); chunks 1..3 and all of v are [c, 128, KC, SC]
    # (dense per chunk: whole-chunk DMAs are linear).
    def _x0arr(x):
        a = x.T.astype(md).reshape(KC, 128, NSC, SC)
        b = a[:, :, 0, :].reshape(KC // 2, 2, 128, SC).transpose(0, 2, 1, 3)
        return np.ascontiguousarray(b)

    def _xRarr(x, c0):
        a = x.T.astype(md).reshape(KC, 128, NSC, SC).transpose(2, 1, 0, 3)
        return np.ascontiguousarray(a[c0:])

    xq0T = [_x0arr(q[b]) for b in range(2)]
    xk0T = [_x0arr(k[b]) for b in range(2)]
    xqRT = [_xRarr(q[b], 1) for b in range(2)]
    xkRT = [_xRarr(k[b], 1) for b in range(2)]
    xvT = [_xRarr(v[b], 0) for b in range(2)]
    in_maps = []
    for c in range(8):
        b, g = c // 4, c % 4
        sl = slice(DL * g, DL * (g + 1))
        in_maps.append(
            {
                "xq0T": xq0T[b],
                "xk0T": xk0T[b],
                "xqRT": xqRT[b],
                "xkRT": xkRT[b],
                "xvT": xvT[b],
                "wqT": _warr((Wq[sl, :].T * f32(0.125)).astype(md), DL),
                "wkT": _warr(Wk[sl, :].T.astype(md), DL),
                "wvT": _warr(Wv[sl, :].T.astype(md), DL),
                "woT": _warr(Wo[:, sl].T.astype(md), D),
                "bqd": np.ascontiguousarray((bq[sl] * f32(0.125)).reshape(2, 128).T),
                "bkd": np.ascontiguousarray(bk[sl].reshape(2, 128).T),
                "maskd": mask,
            }
        )
    return in_maps


def kernel(q, k, v, Wq, bq, Wk, bk, Wv, bv, Wo, bo):
    from concourse.bass_utils import run_bass_kernel_spmd

    args = [np.asarray(a, dtype=np.float32) for a in (q, k, v, Wq, bq, Wk, bk, Wv, bv, Wo, bo)]
    q, k, v, Wq, bq, Wk, bk, Wv, bv, Wo, bo = args
    nc = _get_nc()
    in_maps = make_in_maps(q, k, v, Wq, bq, Wk, bk, Wv, bv, Wo, bo)
    tmpdir = os.environ.get("BASS_KERNEL_TMPDIR") or None
    res = run_bass_kernel_spmd(nc, in_maps, list(range(8)), trace=TRACE, tmpdir=tmpdir)
    if TRACE and res.exec_time_ns is not None:
        print(f"HW exec time: {res.exec_time_ns} ns")
        print(f"HW exec time mean: {res.mean_exec_time_ns} ns")
    out = np.zeros((2, S, D), np.float32)
    for c in range(8):
        out[c // 4] += res.results[c]["out"].astype(np.float32)
    out += (bv @ Wo.T + bo)[None, None, :]
    return out
